# revision 5
# baseline (speedup 1.0000x reference)
"""Trainium2 Bass kernel for nn_MetaController (GRU meta-controller).

Architecture (B=4, N=512, D=512, H=1024, R=16):
  - 2 GRUs (action-proposer, switching-unit) over N=512 sequential steps
  - reparameterized sampling, sigmoid beta gate
  - gated linear scan over time (tensor_tensor_scan)
  - decoder MLP -> low-rank hypernetwork; algebraic simplifications:
      * w2-half of dec_w2 only appears as sum over d -> pre-reduced on host to [16,H]
      * y[d] = sum_r w1[d,r] * s2[r] computed via r-major GEMM + DVE contraction

Sharding (8 cores, identical SPMD program, per-core *data* differs):
  core c: batch b=c//2, role=c%2 (0: ap-GRU + r-half 0, 1: su-GRU + r-half 1)
  - each core runs one GRU chain (B=1) -- the recurrence is LS-bandwidth-bound,
    so B=1 per core costs the same as B=4 and uses all 8 cores
  - pairwise AllGather exchanges the two GRU outputs within a (batch) pair
  - both pair cores compute sampled/beta/scan for the full 512 tokens
  - decoder W2a GEMM sharded by rank-half (8 of 16 r per core), partial y
    summed with a pairwise AllReduce; even cores' output is used by the host
"""

import sys

sys.path.insert(0, "/opt/trn_rl_repo")

import numpy as np

import concourse.bass as bass
import concourse.tile as tile
from concourse import bacc, mybir
from concourse.bass_utils import run_bass_kernel_spmd

F32 = mybir.dt.float32
BF16 = mybir.dt.bfloat16
AF = mybir.ActivationFunctionType
ALU = mybir.AluOpType

B, N, D = 4, 512, 512
G = 3 * D            # 1536 gate width
H = 1024             # decoder hidden
R = 16               # low rank
P = 128
DC = D // P          # 4 d-chunks
GC = G // P          # 12 gate chunks
HC = H // P          # 8 hidden chunks
RH = R // 2          # 8 ranks per core
NCORES = 8
PAIRS = [[2 * i, 2 * i + 1] for i in range(4)]

# precision knobs
GRU_DT = F32         # dtype of W_hh stationary + h moving in the recurrence
W2A_DT = F32         # dtype of the big decoder GEMM (lhsT + rhs)


def _build_program(nsteps=N):
    nc = bacc.Bacc("TRN2", target_bir_lowering=False, debug=False,
                   num_devices=NCORES)

    def din(name, shape, dt=F32):
        return nc.dram_tensor(name, list(shape), dt, kind="ExternalInput").ap()

    xT_d = din("xT", [D, N])                    # residual[b].T
    noiseT_d = din("noiseT", [D, N])
    wihT_d = din("wihT", [D, G])                # this core's GRU W_ih^T
    whhT_d = din("whhT", [D, G], GRU_DT)        # W_hh^T (LS-streamed)
    xbias_d = din("xbias", [P, GC])             # b_ih (+b_hh for r,z) chunk-major
    bhhn_d = din("bhhn", [P, DC])               # b_hh n-part
    aowT_d = din("aowT", [D, 2 * D])            # ap_out_w^T
    bwrep_d = din("bwrep", [D, P])              # beta_w^T replicated to 128 cols
    dw1T_d = din("dw1T", [D, H])                # dec_w1^T
    db1_d = din("db1", [P, HC])
    w2a_d = din("w2a", [RH * DC, P, HC, P], W2A_DT)  # pre-tiled lhsT chunks
    b2a_d = din("b2a", [P, RH * DC])
    w2sT_d = din("w2sT", [H, R])                # pre-reduced w2-half
    b2s_d = din("b2s", [R, 1])
    sel_d = din("sel", [P, RH * P])             # padded row-selectors for r bcast

    outT_d = nc.dram_tensor("outT", [P, DC, N], F32, kind="ExternalOutput").ap()

    with tile.TileContext(nc) as tc:
        from contextlib import ExitStack
        with ExitStack() as ctx:
            perm = ctx.enter_context(tc.tile_pool(name="perm", bufs=1))
            ppb = ctx.enter_context(tc.tile_pool(name="ppb", bufs=3, space="PSUM"))
            pps = ctx.enter_context(tc.tile_pool(name="pps", bufs=2, space="PSUM"))
            dram = ctx.enter_context(tc.tile_pool(name="dram", bufs=1, space="DRAM"))

            xT_sb = perm.tile([P, DC, N], F32)
            nc.sync.dma_start(xT_sb[:], xT_d.rearrange("(k p) t -> p k t", p=P))
            h_sb = perm.tile([P, DC, N], F32)
            gated_sb = perm.tile([P, DC, N], F32)
            xbias_sb = perm.tile([P, GC], F32)
            nc.sync.dma_start(xbias_sb[:], xbias_d[:])
            bhhn_sb = perm.tile([P, DC], F32)
            nc.sync.dma_start(bhhn_sb[:], bhhn_d[:])

            # ---------------- phase 1+2: xp GEMM, GRU recurrence ----------------
            with tc.tile_pool(name="gru", bufs=1) as pg:
                whh_sb = pg.tile([P, DC, G], GRU_DT)
                nc.sync.dma_start(whh_sb[:], whhT_d.rearrange("(k p) g -> p k g", p=P))
                xp_sb = pg.tile([P, GC, N], F32)

                with tc.tile_pool(name="ph1", bufs=1) as p1:
                    wih_sb = p1.tile([P, DC, G], F32)
                    nc.sync.dma_start(wih_sb[:],
                                      wihT_d.rearrange("(k p) g -> p k g", p=P))
                    for m in range(GC):
                        ps = ppb.tile([P, N], F32, name="ps_xp", tag="psbig")
                        for k in range(DC):
                            nc.tensor.matmul(ps[:], lhsT=wih_sb[:, k, m * P:(m + 1) * P],
                                             rhs=xT_sb[:, k, :],
                                             start=(k == 0), stop=(k == DC - 1))
                        nc.scalar.activation(xp_sb[:, m, :], ps[:], AF.Identity,
                                             bias=xbias_sb[:, m:m + 1])

                # ---- recurrence ----
                if GRU_DT != F32:
                    h16 = pg.tile([P, DC, N], GRU_DT)

                def h_rhs(k, t):
                    if GRU_DT == F32:
                        return h_sb[:, k, t:t + 1]
                    return h16[:, k, t:t + 1]

                # step 0: hp = 0
                rz0 = pg.tile([P, 8], F32, name="rz_s", bufs=2)
                nc.scalar.activation(rz0[:], xp_sb[:, 0:8, 0], AF.Sigmoid)
                t1 = pg.tile([P, DC], F32, name="t1_s", bufs=2)
                nc.vector.tensor_tensor(t1[:], rz0[:, 0:4], bhhn_sb[:], ALU.mult)
                nc.vector.tensor_tensor(t1[:], t1[:], xp_sb[:, 8:12, 0], ALU.add)
                n0 = pg.tile([P, DC], F32, name="n_s", bufs=2)
                nc.scalar.activation(n0[:], t1[:], AF.Tanh)
                # h0 = (1-z)*n = n - z*n
                tz = pg.tile([P, DC], F32, name="tz_s", bufs=2)
                nc.vector.tensor_tensor(tz[:], rz0[:, 4:8], n0[:], ALU.mult)
                nc.vector.tensor_tensor(h_sb[:, :, 0], n0[:], tz[:], ALU.subtract)
                if GRU_DT != F32:
                    nc.scalar.activation(h16[:, :, 0], h_sb[:, :, 0], AF.Copy)

                for t in range(1, nsteps):
                    psA = pps.tile([P, 8], F32, name="psA")
                    psB = pps.tile([P, DC], F32, name="psB")
                    for j in range(8):
                        for k in range(DC):
                            nc.tensor.matmul(psA[:, j:j + 1],
                                             lhsT=whh_sb[:, k, j * P:(j + 1) * P],
                                             rhs=h_rhs(k, t - 1),
                                             start=(k == 0), stop=(k == DC - 1))
                    for j in range(8, GC):
                        for k in range(DC):
                            nc.tensor.matmul(psB[:, j - 8:j - 7],
                                             lhsT=whh_sb[:, k, j * P:(j + 1) * P],
                                             rhs=h_rhs(k, t - 1),
                                             start=(k == 0), stop=(k == DC - 1))
                    rzs = pg.tile([P, 8], F32, name="rz_s", bufs=2)
                    nc.vector.tensor_tensor(rzs[:], psA[:], xp_sb[:, 0:8, t], ALU.add)
                    nc.scalar.activation(rzs[:], rzs[:], AF.Sigmoid)
                    t1 = pg.tile([P, DC], F32, name="t1_s", bufs=2)
                    nc.vector.tensor_tensor(t1[:], psB[:], bhhn_sb[:], ALU.add)
                    nc.vector.tensor_tensor(t1[:], rzs[:, 0:4], t1[:], ALU.mult)
                    nc.vector.tensor_tensor(t1[:], t1[:], xp_sb[:, 8:12, t], ALU.add)
                    nn = pg.tile([P, DC], F32, name="n_s", bufs=2)
                    nc.scalar.activation(nn[:], t1[:], AF.Tanh)
                    # h = n + z*(h_prev - n)
                    tz = pg.tile([P, DC], F32, name="tz_s", bufs=2)
                    nc.vector.tensor_tensor(tz[:], h_sb[:, :, t - 1], nn[:], ALU.subtract)
                    nc.vector.tensor_tensor(tz[:], rzs[:, 4:8], tz[:], ALU.mult)
                    nc.vector.tensor_tensor(h_sb[:, :, t], nn[:], tz[:], ALU.add)
                    if GRU_DT != F32:
                        nc.scalar.activation(h16[:, :, t], h_sb[:, :, t], AF.Copy)

            # ---------------- phase 3: pair AllGather of h ----------------
            hT_dr = dram.tile([P, DC, N], F32)
            hpair_dr = dram.tile([2, P, DC, N], F32)
            nc.sync.dma_start(hT_dr[:], h_sb[:])
            nc.gpsimd.collective_compute(
                "AllGather", ALU.bypass, replica_groups=PAIRS,
                ins=[hT_dr.opt()], outs=[hpair_dr.opt()])

            # ---------------- phase 4: sampled, beta, gated scan ----------------
            with tc.tile_pool(name="ph4", bufs=1) as p4, \
                 tc.tile_pool(name="wch", bufs=4) as wch:
                aph_sb = p4.tile([P, DC, N], F32)
                suh_sb = p4.tile([P, DC, N], F32)
                nc.sync.dma_start(aph_sb[:], hpair_dr[0])
                nc.sync.dma_start(suh_sb[:], hpair_dr[1])
                noise_sb = p4.tile([P, DC, N], F32)
                nc.sync.dma_start(noise_sb[:],
                                  noiseT_d.rearrange("(k p) t -> p k t", p=P))
                bwrep_sb = p4.tile([P, DC, P], F32)
                nc.sync.dma_start(bwrep_sb[:],
                                  bwrep_d.rearrange("(k p) m -> p k m", p=P))

                beta_sb = p4.tile([P, N], F32)
                psb = ppb.tile([P, N], F32, name="ps_beta", tag="psbig")
                for k in range(DC):
                    nc.tensor.matmul(psb[:], lhsT=bwrep_sb[:, k, :],
                                     rhs=suh_sb[:, k, :],
                                     start=(k == 0), stop=(k == DC - 1))
                nc.scalar.activation(beta_sb[:], psb[:], AF.Sigmoid)
                forget_sb = p4.tile([P, N], F32)
                nc.scalar.activation(forget_sb[:], beta_sb[:], AF.Identity,
                                     bias=1.0, scale=-1.0)

                samp_sb = p4.tile([P, DC, N], F32)
                for k in range(DC):
                    psm = ppb.tile([P, N], F32, name="ps_mean", tag="psbig")
                    psv = ppb.tile([P, N], F32, name="ps_lv", tag="psbig")
                    for kk in range(DC):
                        mch = wch.tile([P, P], F32, name="aow_m")
                        nc.sync.dma_start(
                            mch[:], aowT_d[kk * P:(kk + 1) * P, k * P:(k + 1) * P])
                        nc.tensor.matmul(psm[:], lhsT=mch[:], rhs=aph_sb[:, kk, :],
                                         start=(kk == 0), stop=(kk == DC - 1))
                    for kk in range(DC):
                        vch = wch.tile([P, P], F32, name="aow_v")
                        nc.sync.dma_start(
                            vch[:], aowT_d[kk * P:(kk + 1) * P,
                                           (DC + k) * P:(DC + k + 1) * P])
                        nc.tensor.matmul(psv[:], lhsT=vch[:], rhs=aph_sb[:, kk, :],
                                         start=(kk == 0), stop=(kk == DC - 1))
                    std = p4.tile([P, N], F32, name="std_t", bufs=2)
                    nc.scalar.activation(std[:], psv[:], AF.Exp, scale=0.5)
                    nc.vector.tensor_tensor(std[:], noise_sb[:, k, :], std[:], ALU.mult)
                    nc.vector.tensor_tensor(samp_sb[:, k, :], psm[:], std[:], ALU.add)
                    # u = forget * sampled (in place), then scan
                    nc.vector.tensor_tensor(samp_sb[:, k, :], samp_sb[:, k, :],
                                            forget_sb[:], ALU.mult)
                    nc.vector.tensor_tensor_scan(gated_sb[:, k, :], beta_sb[:],
                                                 samp_sb[:, k, :], 0.0,
                                                 ALU.mult, ALU.add)

            # ---------------- phase 5: decoder ----------------
            y_dr = dram.tile([P, DC, N], F32)
            ysum_dr = dram.tile([P, DC, N], F32)
            with tc.tile_pool(name="ph5", bufs=1) as p5, \
                 tc.tile_pool(name="w2p", bufs=3) as w2p, \
                 tc.tile_pool(name="s2p", bufs=2) as s2p:
                db1_sb = p5.tile([P, HC], F32)
                nc.sync.dma_start(db1_sb[:], db1_d[:])
                hid_sb = p5.tile([P, HC, N], W2A_DT)
                with tc.tile_pool(name="dw1", bufs=4) as dw1p:
                    for m in range(HC):
                        ps = ppb.tile([P, N], F32, name="ps_hid", tag="psbig")
                        for k in range(DC):
                            wc = dw1p.tile([P, P], F32, name="dw1_c")
                            nc.sync.dma_start(
                                wc[:], dw1T_d[k * P:(k + 1) * P, m * P:(m + 1) * P])
                            nc.tensor.matmul(ps[:], lhsT=wc[:], rhs=gated_sb[:, k, :],
                                             start=(k == 0), stop=(k == DC - 1))
                        nc.scalar.activation(hid_sb[:, m, :], ps[:], AF.Silu,
                                             bias=db1_sb[:, m:m + 1])

                # s2 row vector [16, N] -> zero-padded to 128 partitions
                w2sT_sb = p5.tile([P, HC, R], W2A_DT)
                nc.sync.dma_start(w2sT_sb[:],
                                  w2sT_d.rearrange("(k p) r -> p k r", p=P))
                b2s_sb = p5.tile([R, 1], F32)
                nc.sync.dma_start(b2s_sb[:], b2s_d[:])
                s2big = p5.tile([P, N], W2A_DT)
                nc.vector.memset(s2big[:], 0.0)
                ps2 = ppb.tile([R, N], F32, name="ps_s2", tag="psbig")
                for kk in range(HC):
                    nc.tensor.matmul(ps2[:], lhsT=w2sT_sb[:, kk, :],
                                     rhs=hid_sb[:, kk, :],
                                     start=(kk == 0), stop=(kk == HC - 1))
                nc.scalar.activation(s2big[0:R, :], ps2[:], AF.Identity,
                                     bias=b2s_sb[:])

                sel_sb = p5.tile([P, RH * P], W2A_DT)
                nc.sync.dma_start(sel_sb[:], sel_d[:])
                b2a_sb = p5.tile([P, RH * DC], F32)
                nc.sync.dma_start(b2a_sb[:], b2a_d[:])

                y_sb = p5.tile([P, DC, N], F32)
                for rl in range(RH):
                    # broadcast s2[r] over 128 partitions via selector matmul
                    pbc = ppb.tile([P, N], F32, name="ps_bc", tag="psbig")
                    nc.tensor.matmul(pbc[:], lhsT=sel_sb[:, rl * P:(rl + 1) * P],
                                     rhs=s2big[:], start=True, stop=True)
                    s2bc = s2p.tile([P, N], F32, name="s2bc")
                    nc.vector.tensor_copy(out=s2bc[:], in_=pbc[:])
                    for db in range(DC):
                        cidx = rl * DC + db
                        w2c = w2p.tile([P, HC, P], W2A_DT, name="w2c")
                        nc.sync.dma_start(w2c[:], w2a_d[cidx])
                        pw = ppb.tile([P, N], F32, name="ps_w1", tag="psbig")
                        for kk in range(HC):
                            nc.tensor.matmul(pw[:], lhsT=w2c[:, kk, :],
                                             rhs=hid_sb[:, kk, :],
                                             start=(kk == 0), stop=(kk == HC - 1))
                        if rl == 0:
                            nc.vector.scalar_tensor_tensor(
                                y_sb[:, db, :], pw[:], b2a_sb[:, cidx:cidx + 1],
                                s2bc[:], ALU.add, ALU.mult)
                        else:
                            tmp = s2p.tile([P, N], F32, name="ytmp")
                            nc.vector.scalar_tensor_tensor(
                                tmp[:], pw[:], b2a_sb[:, cidx:cidx + 1],
                                s2bc[:], ALU.add, ALU.mult)
                            nc.vector.tensor_tensor(y_sb[:, db, :], y_sb[:, db, :],
                                                    tmp[:], ALU.add)

                # pairwise AllReduce of partial y
                nc.sync.dma_start(y_dr[:], y_sb[:])
                nc.gpsimd.collective_compute(
                    "AllReduce", ALU.add, replica_groups=PAIRS,
                    ins=[y_dr.opt()], outs=[ysum_dr.opt()])
                ysum_sb = p5.tile([P, DC, N], F32)
                nc.sync.dma_start(ysum_sb[:], ysum_dr[:])

                out_sb = p5.tile([P, DC, N], F32)
                for k in range(DC):
                    nc.vector.tensor_tensor(out_sb[:, k, :], gated_sb[:, k, :],
                                            ysum_sb[:, k, :], ALU.mult)
                    nc.vector.tensor_tensor(out_sb[:, k, :], out_sb[:, k, :],
                                            xT_sb[:, k, :], ALU.add)
                nc.sync.dma_start(outT_d[:], out_sb[:])

    nc.compile()
    return nc


_PROG = {}


def _get_program(nsteps=N):
    if nsteps not in _PROG:
        _PROG[nsteps] = _build_program(nsteps)
    return _PROG[nsteps]


def _prep_in_maps(inputs):
    f = np.float32
    res = np.asarray(inputs["residual_stream"], f)
    noi = np.asarray(inputs["noise"], f)
    gru_w = {
        0: (np.asarray(inputs["ap_w_ih"], f), np.asarray(inputs["ap_w_hh"], f),
            np.asarray(inputs["ap_b_ih"], f), np.asarray(inputs["ap_b_hh"], f)),
        1: (np.asarray(inputs["su_w_ih"], f), np.asarray(inputs["su_w_hh"], f),
            np.asarray(inputs["su_b_ih"], f), np.asarray(inputs["su_b_hh"], f)),
    }
    aowT = np.ascontiguousarray(np.asarray(inputs["ap_out_w"], f).T)      # [D, 2D]
    bwrep = np.ascontiguousarray(
        np.tile(np.asarray(inputs["beta_w"], f).reshape(D, 1), (1, P)))   # [D, P]
    dw1T = np.ascontiguousarray(np.asarray(inputs["dec_w1"], f).T)        # [D, H]
    db1 = np.ascontiguousarray(
        np.asarray(inputs["dec_b1"], f).reshape(HC, P).T)                 # [P, HC]
    w2 = np.asarray(inputs["dec_w2"], f)                                  # [2DR, H]
    b2 = np.asarray(inputs["dec_b2"], f)                                  # [2DR]
    W2a = w2[:D * R].reshape(D, R, H)                                     # [d, r, h]
    B2a = b2[:D * R].reshape(D, R)
    W2s = w2[D * R:].reshape(D, R, H).sum(axis=0)                         # [R, H]
    b2s = b2[D * R:].reshape(D, R).sum(axis=0).reshape(R, 1)              # [R, 1]
    w2sT = np.ascontiguousarray(W2s.T)                                    # [H, R]

    np_w2a_dt = np.float32 if W2A_DT == F32 else np.dtype("bfloat16")

    in_maps = []
    for c in range(NCORES):
        b, role = c // 2, c % 2
        w_ih, w_hh, b_ih, b_hh = gru_w[role]
        xbias = b_ih + np.concatenate([b_hh[:2 * D], np.zeros(D, f)])
        xbias = np.ascontiguousarray(xbias.reshape(GC, P).T)              # [P, GC]
        bhhn = np.ascontiguousarray(b_hh[2 * D:].reshape(DC, P).T)        # [P, DC]

        # rank-half shard of the w1-part of dec_w2
        rsl = slice(role * RH, (role + 1) * RH)
        sub = W2a[:, rsl, :]                                              # [D, RH, H]
        t = sub.transpose(1, 0, 2).reshape(RH, DC, P, H)                  # [rl,db,m,h]
        w2a_tiled = np.ascontiguousarray(
            t.transpose(0, 1, 3, 2).reshape(RH * DC, HC, P, P)
            .transpose(0, 2, 1, 3))                                       # [cidx,p,kk,m]
        b2a_c = np.zeros((P, RH * DC), f)
        for rl in range(RH):
            for db in range(DC):
                b2a_c[:, rl * DC + db] = B2a[db * P:(db + 1) * P, role * RH + rl]
        sel = np.zeros((P, RH * P), f)
        for rl in range(RH):
            sel[role * RH + rl, rl * P:(rl + 1) * P] = 1.0

        in_maps.append({
            "xT": np.ascontiguousarray(res[b].T),
            "noiseT": np.ascontiguousarray(noi[b].T),
            "wihT": np.ascontiguousarray(w_ih.T),
            "whhT": np.ascontiguousarray(w_hh.T).astype(
                np.float32 if GRU_DT == F32 else np.dtype("bfloat16")),
            "xbias": xbias,
            "bhhn": bhhn,
            "aowT": aowT,
            "bwrep": bwrep,
            "dw1T": dw1T,
            "db1": db1,
            "w2a": w2a_tiled.astype(np_w2a_dt),
            "b2a": b2a_c,
            "w2sT": w2sT.astype(np_w2a_dt),
            "b2s": b2s,
            "sel": sel.astype(np_w2a_dt),
        })
    return in_maps


def kernel(**inputs):
    nc = _get_program()
    in_maps = _prep_in_maps(inputs)
    rr = run_bass_kernel_spmd(nc, in_maps, list(range(NCORES)))
    modified = np.empty((B, N, D), np.float32)
    for b in range(B):
        o = rr.results[2 * b]["outT"]                      # [P, DC, N]
        modified[b] = o.transpose(2, 1, 0).reshape(N, D)
    return modified, np.zeros((), np.float32)


def _install_ntff_shim():
    """The image's antenv lacks axon_hooks; synthesize it and register the
    ctypes-based NTFF profile hook from trn_agent_boot."""
    import types
    if "antenv.axon_hooks" in sys.modules:
        return
    mod = types.ModuleType("antenv.axon_hooks")
    holder = {}
    mod.set_axon_ntff_profile_hook = lambda h: holder.__setitem__("h", h)
    mod.get_axon_ntff_profile_hook = lambda: holder.get("h")
    sys.modules["antenv.axon_hooks"] = mod
    import antenv
    antenv.axon_hooks = mod
    from trn_agent_boot.trn_boot import _ntff_profile_via_ctypes
    mod.set_axon_ntff_profile_hook(
        _ntff_profile_via_ctypes("/opt/axon/libaxon_pjrt.so"))


def profile_once(inputs, trace_kwargs=None, tmpdir=None):
    """Run once with NTFF tracing; returns BassKernelResults with
    exec_time_ns / trace. NTFF + trace artifacts land in tmpdir."""
    import tempfile
    import concourse.bass_utils as bu
    _install_ntff_shim()
    bu.upload_artifacts = lambda d: str(d)  # no bucket in this container
    nc = _get_program()
    in_maps = _prep_in_maps(inputs)
    if tmpdir is None:
        tmpdir = tempfile.mkdtemp(prefix="ntff_")
    rr = run_bass_kernel_spmd(nc, in_maps, list(range(NCORES)), trace=True,
                              tmpdir=tmpdir, trace_kwargs=trace_kwargs or {})
    return rr


if __name__ == "__main__":
    import time
    nsteps = int(sys.argv[1]) if len(sys.argv) > 1 else N
    t0 = time.time()
    nc = _build_program(nsteps)
    print(f"build+compile nsteps={nsteps}:", time.time() - t0)


# revision 11
# speedup vs baseline: 4.9242x; 4.9242x over previous
"""Trainium2 Bass kernel for nn_MetaController (GRU meta-controller).

Architecture (B=4, N=512, D=512, H=1024, R=16):
  - 2 GRUs (action-proposer, switching-unit) over N=512 sequential steps
  - reparameterized sampling, sigmoid beta gate
  - gated linear scan over time (tensor_tensor_scan)
  - decoder MLP -> low-rank hypernetwork; algebraic simplifications:
      * w2-half of dec_w2 only appears as sum over d -> pre-reduced on host to [16,H]
      * y[d] = sum_r w1[d,r] * s2[r] computed via r-major GEMM + DVE contraction

Sharding (8 cores, identical SPMD program, per-core *data* differs):
  core c: batch b=c//2, role=c%2 (0: ap-GRU + r-half 0, 1: su-GRU + r-half 1)
  - each core runs one GRU chain (B=1) -- the recurrence is LS-bandwidth-bound,
    so B=1 per core costs the same as B=4 and uses all 8 cores
  - pairwise AllGather exchanges the two GRU outputs within a (batch) pair
  - both pair cores compute sampled/beta/scan for the full 512 tokens
  - decoder W2a GEMM sharded by rank-half (8 of 16 r per core), partial y
    summed with a pairwise AllReduce; even cores' output is used by the host
"""

import sys

sys.path.insert(0, "/opt/trn_rl_repo")

import numpy as np

import concourse.bass as bass
import concourse.tile as tile
from concourse import bacc, mybir
from concourse.bass_utils import run_bass_kernel_spmd

F32 = mybir.dt.float32
BF16 = mybir.dt.bfloat16
AF = mybir.ActivationFunctionType
ALU = mybir.AluOpType

B, N, D = 4, 512, 512
G = 3 * D            # 1536 gate width
H = 1024             # decoder hidden
R = 16               # low rank
P = 128
DC = D // P          # 4 d-chunks
GC = G // P          # 12 gate chunks
HC = H // P          # 8 hidden chunks
RH = R // 2          # 8 ranks per core
NCORES = 8
PAIRS = [[2 * i, 2 * i + 1] for i in range(4)]

# precision knobs
GRU_MODE = "bf16"    # "f32" | "bf16" | "split" (hi/lo bf16, ~fp32 accuracy)
BIG_F32R = False     # float32r GEMMs: verifier requires f32r-typed producers; off
W2A_DT = F32         # dtype of the big decoder GEMM (lhsT + rhs)

GRU_DT = F32 if GRU_MODE == "f32" else BF16


def _r32(ap):
    """View an fp32 AP as float32r for full-rate PE streaming."""
    return ap.bitcast(mybir.dt.float32r) if BIG_F32R else ap


def _build_program(nsteps=N):
    nc = bacc.Bacc("TRN2", target_bir_lowering=False, debug=False,
                   num_devices=NCORES)

    def din(name, shape, dt=F32):
        return nc.dram_tensor(name, list(shape), dt, kind="ExternalInput").ap()

    xT_d = din("xT", [D, N])                    # residual[b].T
    noiseT_d = din("noiseT", [D, N])
    wihT_d = din("wihT", [D, G])                # this core's GRU W_ih^T
    whhT_d = din("whhT", [D, G], GRU_DT)        # W_hh^T (LS-streamed)
    if GRU_MODE == "split":
        whhlo_d = din("whhLo", [D, G], BF16)    # W_hh^T residual (hi/lo split)
    xbias_d = din("xbias", [P, GC])             # b_ih (+b_hh for r,z) chunk-major
    bhhn_d = din("bhhn", [P, DC])               # b_hh n-part
    aowT_d = din("aowT", [D, 2 * D])            # ap_out_w^T
    bwrep_d = din("bwrep", [D, P])              # beta_w^T replicated to 128 cols
    dw1T_d = din("dw1T", [D, H])                # dec_w1^T
    db1_d = din("db1", [P, HC])
    w2a_d = din("w2a", [RH * DC, P, HC, P], W2A_DT)  # pre-tiled lhsT chunks
    b2a_d = din("b2a", [P, RH * DC])
    w2sT_d = din("w2sT", [H, R])                # pre-reduced w2-half
    b2s_d = din("b2s", [R, 1])
    sel_d = din("sel", [P, RH * P])             # padded row-selectors for r bcast

    outT_d = nc.dram_tensor("outT", [P, DC, N], F32, kind="ExternalOutput").ap()

    with tile.TileContext(nc) as tc:
        from contextlib import ExitStack
        with ExitStack() as ctx:
            perm = ctx.enter_context(tc.tile_pool(name="perm", bufs=1))
            ppb = ctx.enter_context(tc.tile_pool(name="ppb", bufs=3, space="PSUM"))
            pps = ctx.enter_context(tc.tile_pool(name="pps", bufs=2, space="PSUM"))
            dram = ctx.enter_context(tc.tile_pool(name="dram", bufs=1, space="DRAM"))

            xT_sb = perm.tile([P, DC, N], F32)
            nc.sync.dma_start(xT_sb[:], xT_d.rearrange("(k p) t -> p k t", p=P))
            h_sb = perm.tile([P, DC, N], F32)
            gated_sb = perm.tile([P, DC, N], F32)
            xbias_sb = perm.tile([P, GC], F32)
            nc.sync.dma_start(xbias_sb[:], xbias_d[:])
            bhhn_sb = perm.tile([P, DC], F32)
            nc.sync.dma_start(bhhn_sb[:], bhhn_d[:])

            # ---------------- phase 1+2: xp GEMM, GRU recurrence ----------------
            with tc.tile_pool(name="gru", bufs=1) as pg:
                whh_sb = pg.tile([P, DC, G], GRU_DT)
                nc.sync.dma_start(whh_sb[:], whhT_d.rearrange("(k p) g -> p k g", p=P))
                if GRU_MODE == "split":
                    whhlo_sb = pg.tile([P, DC, G], BF16)
                    nc.sync.dma_start(whhlo_sb[:],
                                      whhlo_d.rearrange("(k p) g -> p k g", p=P))
                xp_sb = pg.tile([P, GC, N], F32)

                with tc.tile_pool(name="ph1", bufs=1) as p1:
                    wih_sb = p1.tile([P, DC, G], F32)
                    nc.sync.dma_start(wih_sb[:],
                                      wihT_d.rearrange("(k p) g -> p k g", p=P))
                    for m in range(GC):
                        ps = ppb.tile([P, N], F32, name="ps_xp", tag="psbig")
                        for k in range(DC):
                            nc.tensor.matmul(ps[:],
                                             lhsT=_r32(wih_sb[:, k, m * P:(m + 1) * P]),
                                             rhs=_r32(xT_sb[:, k, :]),
                                             start=(k == 0), stop=(k == DC - 1))
                        nc.scalar.activation(xp_sb[:, m, :], ps[:], AF.Identity,
                                             bias=xbias_sb[:, m:m + 1])

                # ---- recurrence ----
                if GRU_MODE != "f32":
                    h16 = pg.tile([P, DC, N], BF16)
                if GRU_MODE == "split":
                    hlo16 = pg.tile([P, DC, N], BF16)

                def emit_h_casts(t):
                    """After h_sb[:, :, t] is written, produce the bf16 views."""
                    if GRU_MODE == "f32":
                        return
                    nc.scalar.activation(h16[:, :, t], h_sb[:, :, t], AF.Copy)
                    if GRU_MODE == "split":
                        lo = pg.tile([P, DC], F32, name="hlo_s", bufs=2)
                        nc.vector.tensor_tensor(lo[:], h_sb[:, :, t], h16[:, :, t],
                                                ALU.subtract)
                        nc.scalar.activation(hlo16[:, :, t], lo[:], AF.Copy)

                def emit_gate_mms(ps_col, j, t):
                    """Accumulate hp for gate chunk j at step t into psum col."""
                    gsl = slice(j * P, (j + 1) * P)
                    pairs = []
                    for k in range(DC):
                        if GRU_MODE == "f32":
                            pairs.append((whh_sb[:, k, gsl], h_sb[:, k, t - 1:t]))
                        else:
                            pairs.append((whh_sb[:, k, gsl], h16[:, k, t - 1:t]))
                            if GRU_MODE == "split":
                                pairs.append((whh_sb[:, k, gsl],
                                              hlo16[:, k, t - 1:t]))
                                pairs.append((whhlo_sb[:, k, gsl],
                                              h16[:, k, t - 1:t]))
                    for i, (lw, rh) in enumerate(pairs):
                        nc.tensor.matmul(ps_col, lhsT=lw, rhs=rh,
                                         start=(i == 0), stop=(i == len(pairs) - 1))

                # step 0: hp = 0
                rz0 = pg.tile([P, 8], F32, name="rz_s", bufs=2)
                nc.scalar.activation(rz0[:], xp_sb[:, 0:8, 0], AF.Sigmoid)
                t1 = pg.tile([P, DC], F32, name="t1_s", bufs=2)
                nc.vector.tensor_tensor(t1[:], rz0[:, 0:4], bhhn_sb[:], ALU.mult)
                nc.vector.tensor_tensor(t1[:], t1[:], xp_sb[:, 8:12, 0], ALU.add)
                n0 = pg.tile([P, DC], F32, name="n_s", bufs=2)
                nc.scalar.activation(n0[:], t1[:], AF.Tanh)
                # h0 = (1-z)*n = n - z*n
                tz = pg.tile([P, DC], F32, name="tz_s", bufs=2)
                nc.vector.tensor_tensor(tz[:], rz0[:, 4:8], n0[:], ALU.mult)
                nc.vector.tensor_tensor(h_sb[:, :, 0], n0[:], tz[:], ALU.subtract)
                emit_h_casts(0)

                for t in range(1, nsteps):
                    psA = pps.tile([P, 8], F32, name="psA")
                    psB = pps.tile([P, DC], F32, name="psB")
                    for j in range(8):
                        emit_gate_mms(psA[:, j:j + 1], j, t)
                    for j in range(8, GC):
                        emit_gate_mms(psB[:, j - 8:j - 7], j, t)
                    rzs = pg.tile([P, 8], F32, name="rz_s", bufs=2)
                    nc.vector.tensor_tensor(rzs[:], psA[:], xp_sb[:, 0:8, t], ALU.add)
                    nc.scalar.activation(rzs[:], rzs[:], AF.Sigmoid)
                    t1 = pg.tile([P, DC], F32, name="t1_s", bufs=2)
                    nc.vector.tensor_tensor(t1[:], psB[:], bhhn_sb[:], ALU.add)
                    nc.vector.tensor_tensor(t1[:], rzs[:, 0:4], t1[:], ALU.mult)
                    nc.vector.tensor_tensor(t1[:], t1[:], xp_sb[:, 8:12, t], ALU.add)
                    nn = pg.tile([P, DC], F32, name="n_s", bufs=2)
                    nc.scalar.activation(nn[:], t1[:], AF.Tanh)
                    # h = n + z*(h_prev - n)
                    tz = pg.tile([P, DC], F32, name="tz_s", bufs=2)
                    nc.vector.tensor_tensor(tz[:], h_sb[:, :, t - 1], nn[:], ALU.subtract)
                    nc.vector.tensor_tensor(tz[:], rzs[:, 4:8], tz[:], ALU.mult)
                    nc.vector.tensor_tensor(h_sb[:, :, t], nn[:], tz[:], ALU.add)
                    emit_h_casts(t)

            # ---------------- phase 3: pair AllGather of h ----------------
            hT_dr = dram.tile([P, DC, N], F32)
            hpair_dr = dram.tile([2, P, DC, N], F32)
            nc.sync.dma_start(hT_dr[:], h_sb[:])
            nc.gpsimd.collective_compute(
                "AllGather", ALU.bypass, replica_groups=PAIRS,
                ins=[hT_dr.opt()], outs=[hpair_dr.opt()])

            # ---------------- phase 4: sampled, beta, gated scan ----------------
            with tc.tile_pool(name="ph4", bufs=1) as p4, \
                 tc.tile_pool(name="wch", bufs=4) as wch:
                aph_sb = p4.tile([P, DC, N], F32)
                suh_sb = p4.tile([P, DC, N], F32)
                nc.sync.dma_start(aph_sb[:], hpair_dr[0])
                nc.sync.dma_start(suh_sb[:], hpair_dr[1])
                noise_sb = p4.tile([P, DC, N], F32)
                nc.sync.dma_start(noise_sb[:],
                                  noiseT_d.rearrange("(k p) t -> p k t", p=P))
                bwrep_sb = p4.tile([P, DC, P], F32)
                nc.sync.dma_start(bwrep_sb[:],
                                  bwrep_d.rearrange("(k p) m -> p k m", p=P))

                beta_sb = p4.tile([P, N], F32)
                psb = ppb.tile([P, N], F32, name="ps_beta", tag="psbig")
                for k in range(DC):
                    nc.tensor.matmul(psb[:], lhsT=_r32(bwrep_sb[:, k, :]),
                                     rhs=_r32(suh_sb[:, k, :]),
                                     start=(k == 0), stop=(k == DC - 1))
                nc.scalar.activation(beta_sb[:], psb[:], AF.Sigmoid)
                forget_sb = p4.tile([P, N], F32)
                nc.scalar.activation(forget_sb[:], beta_sb[:], AF.Identity,
                                     bias=1.0, scale=-1.0)

                samp_sb = p4.tile([P, DC, N], F32)
                for k in range(DC):
                    psm = ppb.tile([P, N], F32, name="ps_mean", tag="psbig")
                    psv = ppb.tile([P, N], F32, name="ps_lv", tag="psbig")
                    for kk in range(DC):
                        mch = wch.tile([P, P], F32, name="aow_m")
                        nc.sync.dma_start(
                            mch[:], aowT_d[kk * P:(kk + 1) * P, k * P:(k + 1) * P])
                        nc.tensor.matmul(psm[:], lhsT=_r32(mch[:]),
                                         rhs=_r32(aph_sb[:, kk, :]),
                                         start=(kk == 0), stop=(kk == DC - 1))
                    for kk in range(DC):
                        vch = wch.tile([P, P], F32, name="aow_v")
                        nc.sync.dma_start(
                            vch[:], aowT_d[kk * P:(kk + 1) * P,
                                           (DC + k) * P:(DC + k + 1) * P])
                        nc.tensor.matmul(psv[:], lhsT=_r32(vch[:]),
                                         rhs=_r32(aph_sb[:, kk, :]),
                                         start=(kk == 0), stop=(kk == DC - 1))
                    std = p4.tile([P, N], F32, name="std_t", bufs=2)
                    nc.scalar.activation(std[:], psv[:], AF.Exp, scale=0.5)
                    nc.vector.tensor_tensor(std[:], noise_sb[:, k, :], std[:], ALU.mult)
                    nc.vector.tensor_tensor(samp_sb[:, k, :], psm[:], std[:], ALU.add)
                    # u = forget * sampled (in place), then scan
                    nc.vector.tensor_tensor(samp_sb[:, k, :], samp_sb[:, k, :],
                                            forget_sb[:], ALU.mult)
                    nc.vector.tensor_tensor_scan(gated_sb[:, k, :], beta_sb[:],
                                                 samp_sb[:, k, :], 0.0,
                                                 ALU.mult, ALU.add)

            # ---------------- phase 5: decoder ----------------
            y_dr = dram.tile([P, DC, N], F32)
            ysum_dr = dram.tile([P, DC, N], F32)
            with tc.tile_pool(name="ph5", bufs=1) as p5, \
                 tc.tile_pool(name="w2p", bufs=3) as w2p, \
                 tc.tile_pool(name="s2p", bufs=2) as s2p:
                db1_sb = p5.tile([P, HC], F32)
                nc.sync.dma_start(db1_sb[:], db1_d[:])
                hid_sb = p5.tile([P, HC, N], W2A_DT)
                with tc.tile_pool(name="dw1", bufs=4) as dw1p:
                    for m in range(HC):
                        ps = ppb.tile([P, N], F32, name="ps_hid", tag="psbig")
                        for k in range(DC):
                            wc = dw1p.tile([P, P], F32, name="dw1_c")
                            nc.sync.dma_start(
                                wc[:], dw1T_d[k * P:(k + 1) * P, m * P:(m + 1) * P])
                            nc.tensor.matmul(ps[:], lhsT=_r32(wc[:]),
                                             rhs=_r32(gated_sb[:, k, :]),
                                             start=(k == 0), stop=(k == DC - 1))
                        nc.scalar.activation(hid_sb[:, m, :], ps[:], AF.Silu,
                                             bias=db1_sb[:, m:m + 1])

                # s2 row vector [16, N] -> zero-padded to 128 partitions
                w2sT_sb = p5.tile([P, HC, R], W2A_DT)
                nc.sync.dma_start(w2sT_sb[:],
                                  w2sT_d.rearrange("(k p) r -> p k r", p=P))
                b2s_sb = p5.tile([R, 1], F32)
                nc.sync.dma_start(b2s_sb[:], b2s_d[:])
                s2big = p5.tile([P, N], W2A_DT)
                nc.vector.memset(s2big[:], 0.0)
                ps2 = ppb.tile([R, N], F32, name="ps_s2", tag="psbig")
                for kk in range(HC):
                    nc.tensor.matmul(ps2[:], lhsT=_r32(w2sT_sb[:, kk, :]),
                                     rhs=_r32(hid_sb[:, kk, :]),
                                     start=(kk == 0), stop=(kk == HC - 1))
                nc.scalar.activation(s2big[0:R, :], ps2[:], AF.Identity,
                                     bias=b2s_sb[:])

                sel_sb = p5.tile([P, RH * P], W2A_DT)
                nc.sync.dma_start(sel_sb[:], sel_d[:])
                b2a_sb = p5.tile([P, RH * DC], F32)
                nc.sync.dma_start(b2a_sb[:], b2a_d[:])

                y_sb = p5.tile([P, DC, N], F32)
                for rl in range(RH):
                    # broadcast s2[r] over 128 partitions via selector matmul
                    pbc = ppb.tile([P, N], F32, name="ps_bc", tag="psbig")
                    nc.tensor.matmul(pbc[:], lhsT=_r32(sel_sb[:, rl * P:(rl + 1) * P]),
                                     rhs=_r32(s2big[:]), start=True, stop=True)
                    s2bc = s2p.tile([P, N], F32, name="s2bc")
                    nc.vector.tensor_copy(out=s2bc[:], in_=pbc[:])
                    for db in range(DC):
                        cidx = rl * DC + db
                        w2c = w2p.tile([P, HC, P], W2A_DT, name="w2c")
                        nc.sync.dma_start(w2c[:], w2a_d[cidx])
                        pw = ppb.tile([P, N], F32, name="ps_w1", tag="psbig")
                        for kk in range(HC):
                            nc.tensor.matmul(pw[:], lhsT=_r32(w2c[:, kk, :]),
                                             rhs=_r32(hid_sb[:, kk, :]),
                                             start=(kk == 0), stop=(kk == HC - 1))
                        if rl == 0:
                            nc.vector.scalar_tensor_tensor(
                                y_sb[:, db, :], pw[:], b2a_sb[:, cidx:cidx + 1],
                                s2bc[:], ALU.add, ALU.mult)
                        else:
                            tmp = s2p.tile([P, N], F32, name="ytmp")
                            nc.vector.scalar_tensor_tensor(
                                tmp[:], pw[:], b2a_sb[:, cidx:cidx + 1],
                                s2bc[:], ALU.add, ALU.mult)
                            nc.vector.tensor_tensor(y_sb[:, db, :], y_sb[:, db, :],
                                                    tmp[:], ALU.add)

                # pairwise AllReduce of partial y
                nc.sync.dma_start(y_dr[:], y_sb[:])
                nc.gpsimd.collective_compute(
                    "AllReduce", ALU.add, replica_groups=PAIRS,
                    ins=[y_dr.opt()], outs=[ysum_dr.opt()])
                ysum_sb = p5.tile([P, DC, N], F32)
                nc.sync.dma_start(ysum_sb[:], ysum_dr[:])

                out_sb = p5.tile([P, DC, N], F32)
                for k in range(DC):
                    nc.vector.tensor_tensor(out_sb[:, k, :], gated_sb[:, k, :],
                                            ysum_sb[:, k, :], ALU.mult)
                    nc.vector.tensor_tensor(out_sb[:, k, :], out_sb[:, k, :],
                                            xT_sb[:, k, :], ALU.add)
                nc.sync.dma_start(outT_d[:], out_sb[:])

    nc.compile()
    return nc


_PROG = {}


def _get_program(nsteps=N):
    if nsteps not in _PROG:
        _PROG[nsteps] = _build_program(nsteps)
    return _PROG[nsteps]


def _prep_in_maps(inputs):
    f = np.float32
    res = np.asarray(inputs["residual_stream"], f)
    noi = np.asarray(inputs["noise"], f)
    gru_w = {
        0: (np.asarray(inputs["ap_w_ih"], f), np.asarray(inputs["ap_w_hh"], f),
            np.asarray(inputs["ap_b_ih"], f), np.asarray(inputs["ap_b_hh"], f)),
        1: (np.asarray(inputs["su_w_ih"], f), np.asarray(inputs["su_w_hh"], f),
            np.asarray(inputs["su_b_ih"], f), np.asarray(inputs["su_b_hh"], f)),
    }
    aowT = np.ascontiguousarray(np.asarray(inputs["ap_out_w"], f).T)      # [D, 2D]
    bwrep = np.ascontiguousarray(
        np.tile(np.asarray(inputs["beta_w"], f).reshape(D, 1), (1, P)))   # [D, P]
    dw1T = np.ascontiguousarray(np.asarray(inputs["dec_w1"], f).T)        # [D, H]
    db1 = np.ascontiguousarray(
        np.asarray(inputs["dec_b1"], f).reshape(HC, P).T)                 # [P, HC]
    w2 = np.asarray(inputs["dec_w2"], f)                                  # [2DR, H]
    b2 = np.asarray(inputs["dec_b2"], f)                                  # [2DR]
    W2a = w2[:D * R].reshape(D, R, H)                                     # [d, r, h]
    B2a = b2[:D * R].reshape(D, R)
    W2s = w2[D * R:].reshape(D, R, H).sum(axis=0)                         # [R, H]
    b2s = b2[D * R:].reshape(D, R).sum(axis=0).reshape(R, 1)              # [R, 1]
    w2sT = np.ascontiguousarray(W2s.T)                                    # [H, R]

    np_w2a_dt = np.float32 if W2A_DT == F32 else np.dtype("bfloat16")

    in_maps = []
    for c in range(NCORES):
        b, role = c // 2, c % 2
        w_ih, w_hh, b_ih, b_hh = gru_w[role]
        xbias = b_ih + np.concatenate([b_hh[:2 * D], np.zeros(D, f)])
        xbias = np.ascontiguousarray(xbias.reshape(GC, P).T)              # [P, GC]
        bhhn = np.ascontiguousarray(b_hh[2 * D:].reshape(DC, P).T)        # [P, DC]

        # rank-half shard of the w1-part of dec_w2
        rsl = slice(role * RH, (role + 1) * RH)
        sub = W2a[:, rsl, :]                                              # [D, RH, H]
        t = sub.transpose(1, 0, 2).reshape(RH, DC, P, H)                  # [rl,db,m,h]
        w2a_tiled = np.ascontiguousarray(
            t.transpose(0, 1, 3, 2).reshape(RH * DC, HC, P, P)
            .transpose(0, 2, 1, 3))                                       # [cidx,p,kk,m]
        b2a_c = np.zeros((P, RH * DC), f)
        for rl in range(RH):
            for db in range(DC):
                b2a_c[:, rl * DC + db] = B2a[db * P:(db + 1) * P, role * RH + rl]
        sel = np.zeros((P, RH * P), f)
        for rl in range(RH):
            sel[role * RH + rl, rl * P:(rl + 1) * P] = 1.0

        whhT = np.ascontiguousarray(w_hh.T)
        im = {
            "xT": np.ascontiguousarray(res[b].T),
            "noiseT": np.ascontiguousarray(noi[b].T),
            "wihT": np.ascontiguousarray(w_ih.T),
            "whhT": whhT.astype(
                np.float32 if GRU_MODE == "f32" else np.dtype("bfloat16")),
            "xbias": xbias,
            "bhhn": bhhn,
            "aowT": aowT,
            "bwrep": bwrep,
            "dw1T": dw1T,
            "db1": db1,
            "w2a": w2a_tiled.astype(np_w2a_dt),
            "b2a": b2a_c,
            "w2sT": w2sT.astype(np_w2a_dt),
            "b2s": b2s,
            "sel": sel.astype(np_w2a_dt),
        }
        if GRU_MODE == "split":
            hi = whhT.astype(np.dtype("bfloat16"))
            im["whhLo"] = (whhT - hi.astype(np.float32)).astype(np.dtype("bfloat16"))
        in_maps.append(im)
    return in_maps


def kernel(**inputs):
    nc = _get_program()
    in_maps = _prep_in_maps(inputs)
    rr = run_bass_kernel_spmd(nc, in_maps, list(range(NCORES)))
    modified = np.empty((B, N, D), np.float32)
    for b in range(B):
        o = rr.results[2 * b]["outT"]                      # [P, DC, N]
        modified[b] = o.transpose(2, 1, 0).reshape(N, D)
    return modified, np.zeros((), np.float32)


def _install_ntff_shim():
    """The image's antenv lacks axon_hooks; synthesize it and register the
    ctypes-based NTFF profile hook from trn_agent_boot."""
    import types
    if "antenv.axon_hooks" in sys.modules:
        return
    mod = types.ModuleType("antenv.axon_hooks")
    holder = {}
    mod.set_axon_ntff_profile_hook = lambda h: holder.__setitem__("h", h)
    mod.get_axon_ntff_profile_hook = lambda: holder.get("h")
    sys.modules["antenv.axon_hooks"] = mod
    import antenv
    antenv.axon_hooks = mod
    from trn_agent_boot.trn_boot import _ntff_profile_via_ctypes
    mod.set_axon_ntff_profile_hook(
        _ntff_profile_via_ctypes("/opt/axon/libaxon_pjrt.so"))


def profile_once(inputs, trace_kwargs=None, tmpdir=None):
    """Run once with NTFF tracing; returns BassKernelResults with
    exec_time_ns / trace. NTFF + trace artifacts land in tmpdir."""
    import tempfile
    import concourse.bass_utils as bu
    _install_ntff_shim()
    bu.upload_artifacts = lambda d: str(d)  # no bucket in this container
    nc = _get_program()
    in_maps = _prep_in_maps(inputs)
    if tmpdir is None:
        tmpdir = tempfile.mkdtemp(prefix="ntff_")
    rr = run_bass_kernel_spmd(nc, in_maps, list(range(NCORES)), trace=True,
                              tmpdir=tmpdir, trace_kwargs=trace_kwargs or {})
    return rr


if __name__ == "__main__":
    import time
    nsteps = int(sys.argv[1]) if len(sys.argv) > 1 else N
    t0 = time.time()
    nc = _build_program(nsteps)
    print(f"build+compile nsteps={nsteps}:", time.time() - t0)


# revision 13
# speedup vs baseline: 5.4723x; 1.1113x over previous
"""Trainium2 Bass kernel for nn_MetaController (GRU meta-controller).

Architecture (B=4, N=512, D=512, H=1024, R=16):
  - 2 GRUs (action-proposer, switching-unit) over N=512 sequential steps
  - reparameterized sampling, sigmoid beta gate
  - gated linear scan over time (tensor_tensor_scan)
  - decoder MLP -> low-rank hypernetwork; algebraic simplifications:
      * w2-half of dec_w2 only appears as sum over d -> pre-reduced on host to [16,H]
      * y[d] = sum_r w1[d,r] * s2[r] computed via r-major GEMM + DVE contraction

Sharding (8 cores, identical SPMD program, per-core *data* differs):
  core c: batch b=c//2, role=c%2 (0: ap-GRU + r-half 0, 1: su-GRU + r-half 1)
  - each core runs one GRU chain (B=1) -- the recurrence is LS-bandwidth-bound,
    so B=1 per core costs the same as B=4 and uses all 8 cores
  - pairwise AllGather exchanges the two GRU outputs within a (batch) pair
  - both pair cores compute sampled/beta/scan for the full 512 tokens
  - decoder W2a GEMM sharded by rank-half (8 of 16 r per core), partial y
    summed with a pairwise AllReduce; even cores' output is used by the host
"""

import sys

sys.path.insert(0, "/opt/trn_rl_repo")

import numpy as np

import concourse.bass as bass
import concourse.tile as tile
from concourse import bacc, mybir
from concourse.bass_utils import run_bass_kernel_spmd

F32 = mybir.dt.float32
BF16 = mybir.dt.bfloat16
AF = mybir.ActivationFunctionType
ALU = mybir.AluOpType

B, N, D = 4, 512, 512
G = 3 * D            # 1536 gate width
H = 1024             # decoder hidden
R = 16               # low rank
P = 128
DC = D // P          # 4 d-chunks
GC = G // P          # 12 gate chunks
HC = H // P          # 8 hidden chunks
RH = R // 2          # 8 ranks per core
NCORES = 8
PAIRS = [[2 * i, 2 * i + 1] for i in range(4)]

# precision knobs
GRU_MODE = "bf16"    # "f32" | "bf16" | "split" (hi/lo bf16, ~fp32 accuracy)
BIG_F32R = False     # float32r GEMMs: verifier requires f32r-typed producers; off
W2A_DT = F32         # dtype of the big decoder GEMM (lhsT + rhs)

GRU_DT = F32 if GRU_MODE == "f32" else BF16


def _r32(ap):
    """View an fp32 AP as float32r for full-rate PE streaming."""
    return ap.bitcast(mybir.dt.float32r) if BIG_F32R else ap


def _build_program(nsteps=N):
    nc = bacc.Bacc("TRN2", target_bir_lowering=False, debug=False,
                   num_devices=NCORES)

    def din(name, shape, dt=F32):
        return nc.dram_tensor(name, list(shape), dt, kind="ExternalInput").ap()

    xT_d = din("xT", [D, N])                    # residual[b].T
    noiseT_d = din("noiseT", [D, N])
    wihT_d = din("wihT", [D, G])                # this core's GRU W_ih^T
    whhT_d = din("whhT", [D, G], GRU_DT)        # W_hh^T (LS-streamed)
    if GRU_MODE == "split":
        whhlo_d = din("whhLo", [D, G], BF16)    # W_hh^T residual (hi/lo split)
    xbias_d = din("xbias", [P, GC])             # b_ih (+b_hh for r,z) chunk-major
    bhhn_d = din("bhhn", [P, DC])               # b_hh n-part
    aowT_d = din("aowT", [D, 2 * D])            # ap_out_w^T
    bwrep_d = din("bwrep", [D, P])              # beta_w^T replicated to 128 cols
    dw1T_d = din("dw1T", [D, H])                # dec_w1^T
    db1_d = din("db1", [P, HC])
    w2a_d = din("w2a", [RH * DC, P, HC, P], W2A_DT)  # pre-tiled lhsT chunks
    b2a_d = din("b2a", [P, RH * DC])
    w2sT_d = din("w2sT", [H, R])                # pre-reduced w2-half
    b2s_d = din("b2s", [R, 1])
    sel_d = din("sel", [P, RH * P])             # padded row-selectors for r bcast
    ident_d = din("ident", [P, P], GRU_DT)      # identity for psum xp preload

    outT_d = nc.dram_tensor("outT", [P, DC, N], F32, kind="ExternalOutput").ap()

    with tile.TileContext(nc) as tc:
        from contextlib import ExitStack
        with ExitStack() as ctx:
            perm = ctx.enter_context(tc.tile_pool(name="perm", bufs=1))
            ppb = ctx.enter_context(tc.tile_pool(name="ppb", bufs=3, space="PSUM"))
            pps = ctx.enter_context(tc.tile_pool(name="pps", bufs=2, space="PSUM"))
            dram = ctx.enter_context(tc.tile_pool(name="dram", bufs=1, space="DRAM"))

            xT_sb = perm.tile([P, DC, N], F32)
            nc.sync.dma_start(xT_sb[:], xT_d.rearrange("(k p) t -> p k t", p=P))
            h_sb = perm.tile([P, DC, N], F32)
            gated_sb = perm.tile([P, DC, N], F32)
            xbias_sb = perm.tile([P, GC], F32)
            nc.sync.dma_start(xbias_sb[:], xbias_d[:])
            bhhn_sb = perm.tile([P, DC], F32)
            nc.sync.dma_start(bhhn_sb[:], bhhn_d[:])

            # ---------------- phase 1+2: xp GEMM, GRU recurrence ----------------
            with tc.tile_pool(name="gru", bufs=1) as pg:
                whh_sb = pg.tile([P, DC, G], GRU_DT)
                nc.sync.dma_start(whh_sb[:], whhT_d.rearrange("(k p) g -> p k g", p=P))
                if GRU_MODE == "split":
                    whhlo_sb = pg.tile([P, DC, G], BF16)
                    nc.sync.dma_start(whhlo_sb[:],
                                      whhlo_d.rearrange("(k p) g -> p k g", p=P))
                xp_sb = pg.tile([P, GC, N], F32)

                with tc.tile_pool(name="ph1", bufs=1) as p1:
                    wih_sb = p1.tile([P, DC, G], F32)
                    nc.sync.dma_start(wih_sb[:],
                                      wihT_d.rearrange("(k p) g -> p k g", p=P))
                    for m in range(GC):
                        ps = ppb.tile([P, N], F32, name="ps_xp", tag="psbig")
                        for k in range(DC):
                            nc.tensor.matmul(ps[:],
                                             lhsT=_r32(wih_sb[:, k, m * P:(m + 1) * P]),
                                             rhs=_r32(xT_sb[:, k, :]),
                                             start=(k == 0), stop=(k == DC - 1))
                        nc.scalar.activation(xp_sb[:, m, :], ps[:], AF.Identity,
                                             bias=xbias_sb[:, m:m + 1])

                # ---- recurrence ----
                ident_sb = pg.tile([P, P], GRU_DT)
                nc.sync.dma_start(ident_sb[:], ident_d[:])
                if GRU_MODE != "f32":
                    h16 = pg.tile([P, DC, N], BF16)
                    # bf16 copies of xp (r,z parts) and bhhn for the psum
                    # preload matmuls (identity lhsT, run in the tail shadow)
                    xp16 = pg.tile([P, 8, N], BF16)
                    for m in range(8):
                        nc.scalar.activation(xp16[:, m, :], xp_sb[:, m, :], AF.Copy)
                    bhhn16 = pg.tile([P, DC], BF16)
                    nc.scalar.activation(bhhn16[:], bhhn_sb[:], AF.Copy)
                    xp_pre, bhhn_pre = xp16, bhhn16
                else:
                    xp_pre, bhhn_pre = xp_sb, bhhn_sb
                if GRU_MODE == "split":
                    hlo16 = pg.tile([P, DC, N], BF16)

                def emit_h_casts(t):
                    """After h_sb[:, :, t] is written, produce the bf16 views."""
                    if GRU_MODE == "f32":
                        return
                    nc.scalar.activation(h16[:, :, t], h_sb[:, :, t], AF.Copy)
                    if GRU_MODE == "split":
                        lo = pg.tile([P, DC], F32, name="hlo_s", bufs=2)
                        nc.vector.tensor_tensor(lo[:], h_sb[:, :, t], h16[:, :, t],
                                                ALU.subtract)
                        nc.scalar.activation(hlo16[:, :, t], lo[:], AF.Copy)

                def emit_gate_mms(ps_col, j, t):
                    """Accumulate hp for gate chunk j at step t into psum col."""
                    gsl = slice(j * P, (j + 1) * P)
                    pairs = []
                    for k in range(DC):
                        if GRU_MODE == "f32":
                            pairs.append((whh_sb[:, k, gsl], h_sb[:, k, t - 1:t]))
                        else:
                            pairs.append((whh_sb[:, k, gsl], h16[:, k, t - 1:t]))
                            if GRU_MODE == "split":
                                pairs.append((whh_sb[:, k, gsl],
                                              hlo16[:, k, t - 1:t]))
                                pairs.append((whhlo_sb[:, k, gsl],
                                              h16[:, k, t - 1:t]))
                    for i, (lw, rh) in enumerate(pairs):
                        nc.tensor.matmul(ps_col, lhsT=lw, rhs=rh,
                                         start=False, stop=(i == len(pairs) - 1),
                                         skip_group_check=True)

                # step 0: hp = 0
                rz0 = pg.tile([P, 8], F32, name="rz_s", bufs=2)
                nc.scalar.activation(rz0[:], xp_sb[:, 0:8, 0], AF.Sigmoid)
                t1 = pg.tile([P, DC], F32, name="t1_s", bufs=2)
                nc.vector.tensor_tensor(t1[:], rz0[:, 0:4], bhhn_sb[:], ALU.mult)
                nc.vector.tensor_tensor(t1[:], t1[:], xp_sb[:, 8:12, 0], ALU.add)
                n0 = pg.tile([P, DC], F32, name="n_s", bufs=2)
                nc.scalar.activation(n0[:], t1[:], AF.Tanh)
                # h0 = (1-z)*n = n - z*n
                tz = pg.tile([P, DC], F32, name="tz_s", bufs=2)
                nc.vector.tensor_tensor(tz[:], rz0[:, 4:8], n0[:], ALU.mult)
                nc.vector.tensor_tensor(h_sb[:, :, 0], n0[:], tz[:], ALU.subtract)
                emit_h_casts(0)

                for t in range(1, nsteps):
                    psA = pps.tile([P, 8], F32, name="psA")
                    psB = pps.tile([P, DC], F32, name="psB")
                    # xp / b_hh_n preloads via one wide identity matmul each:
                    # independent of h[t-1], so they run under the previous
                    # step's tail (per-column interleaved groups are broken on
                    # HW; a single start=True covering the bank is correct)
                    nc.tensor.matmul(psA[:, :], lhsT=ident_sb[:],
                                     rhs=xp_pre[:, 0:8, t], start=True, stop=False)
                    nc.tensor.matmul(psB[:, :], lhsT=ident_sb[:],
                                     rhs=bhhn_pre[:, :], start=True, stop=False)
                    for j in range(8):
                        emit_gate_mms(psA[:, j:j + 1], j, t)
                    for j in range(8, GC):
                        emit_gate_mms(psB[:, j - 8:j - 7], j, t)
                    rzs = pg.tile([P, 8], F32, name="rz_s", bufs=2)
                    nc.scalar.activation(rzs[:], psA[:], AF.Sigmoid)
                    t1 = pg.tile([P, DC], F32, name="t1_s", bufs=2)
                    nc.vector.tensor_tensor(t1[:], rzs[:, 0:4], psB[:], ALU.mult)
                    nc.vector.tensor_tensor(t1[:], t1[:], xp_sb[:, 8:12, t], ALU.add)
                    nn = pg.tile([P, DC], F32, name="n_s", bufs=2)
                    nc.scalar.activation(nn[:], t1[:], AF.Tanh)
                    # h = n + z*(h_prev - n); bf16 h16 written first (it alone
                    # gates the next step's matmuls), fp32 state off-path
                    tz = pg.tile([P, DC], F32, name="tz_s", bufs=2)
                    nc.vector.tensor_tensor(tz[:], h_sb[:, :, t - 1], nn[:], ALU.subtract)
                    nc.vector.tensor_tensor(tz[:], rzs[:, 4:8], tz[:], ALU.mult)
                    if GRU_MODE == "f32":
                        nc.vector.tensor_tensor(h_sb[:, :, t], nn[:], tz[:], ALU.add)
                    else:
                        nc.vector.tensor_tensor(h16[:, :, t], nn[:], tz[:], ALU.add)
                        nc.vector.tensor_tensor(h_sb[:, :, t], nn[:], tz[:], ALU.add)
                        if GRU_MODE == "split":
                            lo = pg.tile([P, DC], F32, name="hlo_s", bufs=2)
                            nc.vector.tensor_tensor(lo[:], h_sb[:, :, t],
                                                    h16[:, :, t], ALU.subtract)
                            nc.scalar.activation(hlo16[:, :, t], lo[:], AF.Copy)

            # ---------------- phase 3: pair AllGather of h ----------------
            hT_dr = dram.tile([P, DC, N], F32)
            hpair_dr = dram.tile([2, P, DC, N], F32)
            nc.sync.dma_start(hT_dr[:], h_sb[:])
            nc.gpsimd.collective_compute(
                "AllGather", ALU.bypass, replica_groups=PAIRS,
                ins=[hT_dr.opt()], outs=[hpair_dr.opt()])

            # ---------------- phase 4: sampled, beta, gated scan ----------------
            with tc.tile_pool(name="ph4", bufs=1) as p4, \
                 tc.tile_pool(name="wch", bufs=4) as wch:
                aph_sb = p4.tile([P, DC, N], F32)
                suh_sb = p4.tile([P, DC, N], F32)
                nc.sync.dma_start(aph_sb[:], hpair_dr[0])
                nc.sync.dma_start(suh_sb[:], hpair_dr[1])
                noise_sb = p4.tile([P, DC, N], F32)
                nc.sync.dma_start(noise_sb[:],
                                  noiseT_d.rearrange("(k p) t -> p k t", p=P))
                bwrep_sb = p4.tile([P, DC, P], F32)
                nc.sync.dma_start(bwrep_sb[:],
                                  bwrep_d.rearrange("(k p) m -> p k m", p=P))

                beta_sb = p4.tile([P, N], F32)
                psb = ppb.tile([P, N], F32, name="ps_beta", tag="psbig")
                for k in range(DC):
                    nc.tensor.matmul(psb[:], lhsT=_r32(bwrep_sb[:, k, :]),
                                     rhs=_r32(suh_sb[:, k, :]),
                                     start=(k == 0), stop=(k == DC - 1))
                nc.scalar.activation(beta_sb[:], psb[:], AF.Sigmoid)
                forget_sb = p4.tile([P, N], F32)
                nc.scalar.activation(forget_sb[:], beta_sb[:], AF.Identity,
                                     bias=1.0, scale=-1.0)

                samp_sb = p4.tile([P, DC, N], F32)
                for k in range(DC):
                    psm = ppb.tile([P, N], F32, name="ps_mean", tag="psbig")
                    psv = ppb.tile([P, N], F32, name="ps_lv", tag="psbig")
                    for kk in range(DC):
                        mch = wch.tile([P, P], F32, name="aow_m")
                        nc.sync.dma_start(
                            mch[:], aowT_d[kk * P:(kk + 1) * P, k * P:(k + 1) * P])
                        nc.tensor.matmul(psm[:], lhsT=_r32(mch[:]),
                                         rhs=_r32(aph_sb[:, kk, :]),
                                         start=(kk == 0), stop=(kk == DC - 1))
                    for kk in range(DC):
                        vch = wch.tile([P, P], F32, name="aow_v")
                        nc.sync.dma_start(
                            vch[:], aowT_d[kk * P:(kk + 1) * P,
                                           (DC + k) * P:(DC + k + 1) * P])
                        nc.tensor.matmul(psv[:], lhsT=_r32(vch[:]),
                                         rhs=_r32(aph_sb[:, kk, :]),
                                         start=(kk == 0), stop=(kk == DC - 1))
                    std = p4.tile([P, N], F32, name="std_t", bufs=2)
                    nc.scalar.activation(std[:], psv[:], AF.Exp, scale=0.5)
                    nc.vector.tensor_tensor(std[:], noise_sb[:, k, :], std[:], ALU.mult)
                    nc.vector.tensor_tensor(samp_sb[:, k, :], psm[:], std[:], ALU.add)
                    # u = forget * sampled (in place), then scan
                    nc.vector.tensor_tensor(samp_sb[:, k, :], samp_sb[:, k, :],
                                            forget_sb[:], ALU.mult)
                    nc.vector.tensor_tensor_scan(gated_sb[:, k, :], beta_sb[:],
                                                 samp_sb[:, k, :], 0.0,
                                                 ALU.mult, ALU.add)

            # ---------------- phase 5: decoder ----------------
            y_dr = dram.tile([P, DC, N], F32)
            ysum_dr = dram.tile([P, DC, N], F32)
            with tc.tile_pool(name="ph5", bufs=1) as p5, \
                 tc.tile_pool(name="w2p", bufs=3) as w2p, \
                 tc.tile_pool(name="s2p", bufs=2) as s2p:
                db1_sb = p5.tile([P, HC], F32)
                nc.sync.dma_start(db1_sb[:], db1_d[:])
                hid_sb = p5.tile([P, HC, N], W2A_DT)
                with tc.tile_pool(name="dw1", bufs=4) as dw1p:
                    for m in range(HC):
                        ps = ppb.tile([P, N], F32, name="ps_hid", tag="psbig")
                        for k in range(DC):
                            wc = dw1p.tile([P, P], F32, name="dw1_c")
                            nc.sync.dma_start(
                                wc[:], dw1T_d[k * P:(k + 1) * P, m * P:(m + 1) * P])
                            nc.tensor.matmul(ps[:], lhsT=_r32(wc[:]),
                                             rhs=_r32(gated_sb[:, k, :]),
                                             start=(k == 0), stop=(k == DC - 1))
                        nc.scalar.activation(hid_sb[:, m, :], ps[:], AF.Silu,
                                             bias=db1_sb[:, m:m + 1])

                # s2 row vector [16, N] -> zero-padded to 128 partitions
                w2sT_sb = p5.tile([P, HC, R], W2A_DT)
                nc.sync.dma_start(w2sT_sb[:],
                                  w2sT_d.rearrange("(k p) r -> p k r", p=P))
                b2s_sb = p5.tile([R, 1], F32)
                nc.sync.dma_start(b2s_sb[:], b2s_d[:])
                s2big = p5.tile([P, N], W2A_DT)
                nc.vector.memset(s2big[:], 0.0)
                ps2 = ppb.tile([R, N], F32, name="ps_s2", tag="psbig")
                for kk in range(HC):
                    nc.tensor.matmul(ps2[:], lhsT=_r32(w2sT_sb[:, kk, :]),
                                     rhs=_r32(hid_sb[:, kk, :]),
                                     start=(kk == 0), stop=(kk == HC - 1))
                nc.scalar.activation(s2big[0:R, :], ps2[:], AF.Identity,
                                     bias=b2s_sb[:])

                sel_sb = p5.tile([P, RH * P], W2A_DT)
                nc.sync.dma_start(sel_sb[:], sel_d[:])
                b2a_sb = p5.tile([P, RH * DC], F32)
                nc.sync.dma_start(b2a_sb[:], b2a_d[:])

                y_sb = p5.tile([P, DC, N], F32)
                for rl in range(RH):
                    # broadcast s2[r] over 128 partitions via selector matmul
                    pbc = ppb.tile([P, N], F32, name="ps_bc", tag="psbig")
                    nc.tensor.matmul(pbc[:], lhsT=_r32(sel_sb[:, rl * P:(rl + 1) * P]),
                                     rhs=_r32(s2big[:]), start=True, stop=True)
                    s2bc = s2p.tile([P, N], F32, name="s2bc")
                    nc.vector.tensor_copy(out=s2bc[:], in_=pbc[:])
                    for db in range(DC):
                        cidx = rl * DC + db
                        w2c = w2p.tile([P, HC, P], W2A_DT, name="w2c")
                        nc.sync.dma_start(w2c[:], w2a_d[cidx])
                        pw = ppb.tile([P, N], F32, name="ps_w1", tag="psbig")
                        for kk in range(HC):
                            nc.tensor.matmul(pw[:], lhsT=_r32(w2c[:, kk, :]),
                                             rhs=_r32(hid_sb[:, kk, :]),
                                             start=(kk == 0), stop=(kk == HC - 1))
                        if rl == 0:
                            nc.vector.scalar_tensor_tensor(
                                y_sb[:, db, :], pw[:], b2a_sb[:, cidx:cidx + 1],
                                s2bc[:], ALU.add, ALU.mult)
                        else:
                            tmp = s2p.tile([P, N], F32, name="ytmp")
                            nc.vector.scalar_tensor_tensor(
                                tmp[:], pw[:], b2a_sb[:, cidx:cidx + 1],
                                s2bc[:], ALU.add, ALU.mult)
                            nc.vector.tensor_tensor(y_sb[:, db, :], y_sb[:, db, :],
                                                    tmp[:], ALU.add)

                # pairwise AllReduce of partial y
                nc.sync.dma_start(y_dr[:], y_sb[:])
                nc.gpsimd.collective_compute(
                    "AllReduce", ALU.add, replica_groups=PAIRS,
                    ins=[y_dr.opt()], outs=[ysum_dr.opt()])
                ysum_sb = p5.tile([P, DC, N], F32)
                nc.sync.dma_start(ysum_sb[:], ysum_dr[:])

                out_sb = p5.tile([P, DC, N], F32)
                for k in range(DC):
                    nc.vector.tensor_tensor(out_sb[:, k, :], gated_sb[:, k, :],
                                            ysum_sb[:, k, :], ALU.mult)
                    nc.vector.tensor_tensor(out_sb[:, k, :], out_sb[:, k, :],
                                            xT_sb[:, k, :], ALU.add)
                nc.sync.dma_start(outT_d[:], out_sb[:])

    nc.compile()
    return nc


_PROG = {}


def _get_program(nsteps=N):
    if nsteps not in _PROG:
        _PROG[nsteps] = _build_program(nsteps)
    return _PROG[nsteps]


def _prep_in_maps(inputs):
    f = np.float32
    res = np.asarray(inputs["residual_stream"], f)
    noi = np.asarray(inputs["noise"], f)
    gru_w = {
        0: (np.asarray(inputs["ap_w_ih"], f), np.asarray(inputs["ap_w_hh"], f),
            np.asarray(inputs["ap_b_ih"], f), np.asarray(inputs["ap_b_hh"], f)),
        1: (np.asarray(inputs["su_w_ih"], f), np.asarray(inputs["su_w_hh"], f),
            np.asarray(inputs["su_b_ih"], f), np.asarray(inputs["su_b_hh"], f)),
    }
    aowT = np.ascontiguousarray(np.asarray(inputs["ap_out_w"], f).T)      # [D, 2D]
    bwrep = np.ascontiguousarray(
        np.tile(np.asarray(inputs["beta_w"], f).reshape(D, 1), (1, P)))   # [D, P]
    dw1T = np.ascontiguousarray(np.asarray(inputs["dec_w1"], f).T)        # [D, H]
    db1 = np.ascontiguousarray(
        np.asarray(inputs["dec_b1"], f).reshape(HC, P).T)                 # [P, HC]
    w2 = np.asarray(inputs["dec_w2"], f)                                  # [2DR, H]
    b2 = np.asarray(inputs["dec_b2"], f)                                  # [2DR]
    W2a = w2[:D * R].reshape(D, R, H)                                     # [d, r, h]
    B2a = b2[:D * R].reshape(D, R)
    W2s = w2[D * R:].reshape(D, R, H).sum(axis=0)                         # [R, H]
    b2s = b2[D * R:].reshape(D, R).sum(axis=0).reshape(R, 1)              # [R, 1]
    w2sT = np.ascontiguousarray(W2s.T)                                    # [H, R]

    np_w2a_dt = np.float32 if W2A_DT == F32 else np.dtype("bfloat16")

    in_maps = []
    for c in range(NCORES):
        b, role = c // 2, c % 2
        w_ih, w_hh, b_ih, b_hh = gru_w[role]
        xbias = b_ih + np.concatenate([b_hh[:2 * D], np.zeros(D, f)])
        xbias = np.ascontiguousarray(xbias.reshape(GC, P).T)              # [P, GC]
        bhhn = np.ascontiguousarray(b_hh[2 * D:].reshape(DC, P).T)        # [P, DC]

        # rank-half shard of the w1-part of dec_w2
        rsl = slice(role * RH, (role + 1) * RH)
        sub = W2a[:, rsl, :]                                              # [D, RH, H]
        t = sub.transpose(1, 0, 2).reshape(RH, DC, P, H)                  # [rl,db,m,h]
        w2a_tiled = np.ascontiguousarray(
            t.transpose(0, 1, 3, 2).reshape(RH * DC, HC, P, P)
            .transpose(0, 2, 1, 3))                                       # [cidx,p,kk,m]
        b2a_c = np.zeros((P, RH * DC), f)
        for rl in range(RH):
            for db in range(DC):
                b2a_c[:, rl * DC + db] = B2a[db * P:(db + 1) * P, role * RH + rl]
        sel = np.zeros((P, RH * P), f)
        for rl in range(RH):
            sel[role * RH + rl, rl * P:(rl + 1) * P] = 1.0

        whhT = np.ascontiguousarray(w_hh.T)
        im = {
            "xT": np.ascontiguousarray(res[b].T),
            "noiseT": np.ascontiguousarray(noi[b].T),
            "wihT": np.ascontiguousarray(w_ih.T),
            "whhT": whhT.astype(
                np.float32 if GRU_MODE == "f32" else np.dtype("bfloat16")),
            "xbias": xbias,
            "bhhn": bhhn,
            "aowT": aowT,
            "bwrep": bwrep,
            "dw1T": dw1T,
            "db1": db1,
            "w2a": w2a_tiled.astype(np_w2a_dt),
            "b2a": b2a_c,
            "w2sT": w2sT.astype(np_w2a_dt),
            "b2s": b2s,
            "sel": sel.astype(np_w2a_dt),
            "ident": np.eye(P, dtype=np.float32).astype(
                np.float32 if GRU_MODE == "f32" else np.dtype("bfloat16")),
        }
        if GRU_MODE == "split":
            hi = whhT.astype(np.dtype("bfloat16"))
            im["whhLo"] = (whhT - hi.astype(np.float32)).astype(np.dtype("bfloat16"))
        in_maps.append(im)
    return in_maps


def kernel(**inputs):
    nc = _get_program()
    in_maps = _prep_in_maps(inputs)
    rr = run_bass_kernel_spmd(nc, in_maps, list(range(NCORES)))
    modified = np.empty((B, N, D), np.float32)
    for b in range(B):
        o = rr.results[2 * b]["outT"]                      # [P, DC, N]
        modified[b] = o.transpose(2, 1, 0).reshape(N, D)
    return modified, np.zeros((), np.float32)


def _install_ntff_shim():
    """The image's antenv lacks axon_hooks; synthesize it and register the
    ctypes-based NTFF profile hook from trn_agent_boot."""
    import types
    if "antenv.axon_hooks" in sys.modules:
        return
    mod = types.ModuleType("antenv.axon_hooks")
    holder = {}
    mod.set_axon_ntff_profile_hook = lambda h: holder.__setitem__("h", h)
    mod.get_axon_ntff_profile_hook = lambda: holder.get("h")
    sys.modules["antenv.axon_hooks"] = mod
    import antenv
    antenv.axon_hooks = mod
    from trn_agent_boot.trn_boot import _ntff_profile_via_ctypes
    mod.set_axon_ntff_profile_hook(
        _ntff_profile_via_ctypes("/opt/axon/libaxon_pjrt.so"))


def profile_once(inputs, trace_kwargs=None, tmpdir=None):
    """Run once with NTFF tracing; returns BassKernelResults with
    exec_time_ns / trace. NTFF + trace artifacts land in tmpdir."""
    import tempfile
    import concourse.bass_utils as bu
    _install_ntff_shim()
    bu.upload_artifacts = lambda d: str(d)  # no bucket in this container
    nc = _get_program()
    in_maps = _prep_in_maps(inputs)
    if tmpdir is None:
        tmpdir = tempfile.mkdtemp(prefix="ntff_")
    rr = run_bass_kernel_spmd(nc, in_maps, list(range(NCORES)), trace=True,
                              tmpdir=tmpdir, trace_kwargs=trace_kwargs or {})
    return rr


if __name__ == "__main__":
    import time
    nsteps = int(sys.argv[1]) if len(sys.argv) > 1 else N
    t0 = time.time()
    nc = _build_program(nsteps)
    print(f"build+compile nsteps={nsteps}:", time.time() - t0)


# revision 14
# speedup vs baseline: 5.9289x; 1.0834x over previous
"""Trainium2 Bass kernel for nn_MetaController (GRU meta-controller).

Architecture (B=4, N=512, D=512, H=1024, R=16):
  - 2 GRUs (action-proposer, switching-unit) over N=512 sequential steps
  - reparameterized sampling, sigmoid beta gate
  - gated linear scan over time (tensor_tensor_scan)
  - decoder MLP -> low-rank hypernetwork; algebraic simplifications:
      * w2-half of dec_w2 only appears as sum over d -> pre-reduced on host to [16,H]
      * y[d] = sum_r w1[d,r] * s2[r] computed via r-major GEMM + DVE contraction

Sharding (8 cores, identical SPMD program, per-core *data* differs):
  core c: batch b=c//2, role=c%2 (0: ap-GRU + r-half 0, 1: su-GRU + r-half 1)
  - each core runs one GRU chain (B=1) -- the recurrence is LS-bandwidth-bound,
    so B=1 per core costs the same as B=4 and uses all 8 cores
  - pairwise AllGather exchanges the two GRU outputs within a (batch) pair
  - both pair cores compute sampled/beta/scan for the full 512 tokens
  - decoder W2a GEMM sharded by rank-half (8 of 16 r per core), partial y
    summed with a pairwise AllReduce; even cores' output is used by the host
"""

import sys

sys.path.insert(0, "/opt/trn_rl_repo")

import numpy as np

import concourse.bass as bass
import concourse.tile as tile
from concourse import bacc, mybir
from concourse.bass_utils import run_bass_kernel_spmd

F32 = mybir.dt.float32
BF16 = mybir.dt.bfloat16
AF = mybir.ActivationFunctionType
ALU = mybir.AluOpType

B, N, D = 4, 512, 512
G = 3 * D            # 1536 gate width
H = 1024             # decoder hidden
R = 16               # low rank
P = 128
DC = D // P          # 4 d-chunks
GC = G // P          # 12 gate chunks
HC = H // P          # 8 hidden chunks
RH = R // 2          # 8 ranks per core
NCORES = 8
PAIRS = [[2 * i, 2 * i + 1] for i in range(4)]

# precision knobs
GRU_MODE = "bf16"    # "f32" | "bf16" | "split" (hi/lo bf16, ~fp32 accuracy)
BIG_F32R = False     # float32r GEMMs: verifier requires f32r-typed producers; off
W2A_DT = F32         # dtype of the big decoder GEMM (lhsT + rhs)

GRU_DT = F32 if GRU_MODE == "f32" else BF16


def _r32(ap):
    """View an fp32 AP as float32r for full-rate PE streaming."""
    return ap.bitcast(mybir.dt.float32r) if BIG_F32R else ap


def _build_program(nsteps=N):
    nc = bacc.Bacc("TRN2", target_bir_lowering=False, debug=False,
                   num_devices=NCORES)

    def din(name, shape, dt=F32):
        return nc.dram_tensor(name, list(shape), dt, kind="ExternalInput").ap()

    xT_d = din("xT", [D, N])                    # residual[b].T
    noiseT_d = din("noiseT", [D, N])
    wihT_d = din("wihT", [D, G])                # this core's GRU W_ih^T
    whhT_d = din("whhT", [D, G], GRU_DT)        # W_hh^T (LS-streamed)
    if GRU_MODE == "split":
        whhlo_d = din("whhLo", [D, G], BF16)    # W_hh^T residual (hi/lo split)
    xbias_d = din("xbias", [P, GC])             # b_ih (+b_hh for r,z) chunk-major
    bhhn_d = din("bhhn", [P, DC])               # b_hh n-part
    aowT_d = din("aowT", [D, 2 * D])            # ap_out_w^T
    bwrep_d = din("bwrep", [D, P])              # beta_w^T replicated to 128 cols
    dw1T_d = din("dw1T", [D, H])                # dec_w1^T
    db1_d = din("db1", [P, HC])
    w2a_d = din("w2a", [RH * DC, P, HC, P], W2A_DT)  # pre-tiled lhsT chunks
    b2a_d = din("b2a", [P, RH * DC])
    w2sT_d = din("w2sT", [H, R])                # pre-reduced w2-half
    b2s_d = din("b2s", [R, 1])
    sel_d = din("sel", [P, RH * P])             # padded row-selectors for r bcast
    ident_d = din("ident", [P, P], GRU_DT)      # identity for psum xp preload

    outT_d = nc.dram_tensor("outT", [P, DC, N], F32, kind="ExternalOutput").ap()

    with tile.TileContext(nc) as tc:
        from contextlib import ExitStack
        with ExitStack() as ctx:
            perm = ctx.enter_context(tc.tile_pool(name="perm", bufs=1))
            ppb = ctx.enter_context(tc.tile_pool(name="ppb", bufs=2, space="PSUM"))
            pps = ctx.enter_context(tc.tile_pool(name="pps", bufs=2, space="PSUM"))
            dram = ctx.enter_context(tc.tile_pool(name="dram", bufs=1, space="DRAM"))

            xT_sb = perm.tile([P, DC, N], F32)
            nc.sync.dma_start(xT_sb[:], xT_d.rearrange("(k p) t -> p k t", p=P))
            h_sb = perm.tile([P, DC, N], F32)
            gated_sb = perm.tile([P, DC, N], F32)
            xbias_sb = perm.tile([P, GC], F32)
            nc.sync.dma_start(xbias_sb[:], xbias_d[:])
            bhhn_sb = perm.tile([P, DC], F32)
            nc.sync.dma_start(bhhn_sb[:], bhhn_d[:])

            # ---------------- phase 1+2: xp GEMM, GRU recurrence ----------------
            with tc.tile_pool(name="gru", bufs=1) as pg:
                whh_sb = pg.tile([P, DC, G], GRU_DT)
                nc.sync.dma_start(whh_sb[:], whhT_d.rearrange("(k p) g -> p k g", p=P))
                if GRU_MODE == "split":
                    whhlo_sb = pg.tile([P, DC, G], BF16)
                    nc.sync.dma_start(whhlo_sb[:],
                                      whhlo_d.rearrange("(k p) g -> p k g", p=P))
                xp_sb = pg.tile([P, GC, N], F32)

                with tc.tile_pool(name="ph1", bufs=1) as p1:
                    wih_sb = p1.tile([P, DC, G], F32)
                    nc.sync.dma_start(wih_sb[:],
                                      wihT_d.rearrange("(k p) g -> p k g", p=P))
                    for m in range(GC):
                        ps = ppb.tile([P, N], F32, name="ps_xp", tag="psbig")
                        for k in range(DC):
                            nc.tensor.matmul(ps[:],
                                             lhsT=_r32(wih_sb[:, k, m * P:(m + 1) * P]),
                                             rhs=_r32(xT_sb[:, k, :]),
                                             start=(k == 0), stop=(k == DC - 1))
                        nc.scalar.activation(xp_sb[:, m, :], ps[:], AF.Identity,
                                             bias=xbias_sb[:, m:m + 1])

                # ---- recurrence ----
                ident_sb = pg.tile([P, P], GRU_DT)
                nc.sync.dma_start(ident_sb[:], ident_d[:])
                if GRU_MODE != "f32":
                    h16 = pg.tile([P, DC, N], BF16)
                    # bf16 copies of xp (r,z parts) and bhhn for the psum
                    # preload matmuls (identity lhsT, run in the tail shadow)
                    xp16 = pg.tile([P, 8, N], BF16)
                    for m in range(8):
                        nc.scalar.activation(xp16[:, m, :], xp_sb[:, m, :], AF.Copy)
                    bhhn16 = pg.tile([P, DC], BF16)
                    nc.scalar.activation(bhhn16[:], bhhn_sb[:], AF.Copy)
                    xp_pre, bhhn_pre = xp16, bhhn16
                else:
                    xp_pre, bhhn_pre = xp_sb, bhhn_sb
                if GRU_MODE == "split":
                    hlo16 = pg.tile([P, DC, N], BF16)

                def emit_h_casts(t):
                    """After h_sb[:, :, t] is written, produce the bf16 views."""
                    if GRU_MODE == "f32":
                        return
                    nc.scalar.activation(h16[:, :, t], h_sb[:, :, t], AF.Copy)
                    if GRU_MODE == "split":
                        lo = pg.tile([P, DC], F32, name="hlo_s", bufs=2)
                        nc.vector.tensor_tensor(lo[:], h_sb[:, :, t], h16[:, :, t],
                                                ALU.subtract)
                        nc.scalar.activation(hlo16[:, :, t], lo[:], AF.Copy)

                def emit_gate_mms(ps_col, j, t):
                    """Accumulate hp for gate chunk j at step t into psum col."""
                    gsl = slice(j * P, (j + 1) * P)
                    pairs = []
                    for k in range(DC):
                        if GRU_MODE == "f32":
                            pairs.append((whh_sb[:, k, gsl], h_sb[:, k, t - 1:t]))
                        else:
                            pairs.append((whh_sb[:, k, gsl], h16[:, k, t - 1:t]))
                            if GRU_MODE == "split":
                                pairs.append((whh_sb[:, k, gsl],
                                              hlo16[:, k, t - 1:t]))
                                pairs.append((whhlo_sb[:, k, gsl],
                                              h16[:, k, t - 1:t]))
                    for i, (lw, rh) in enumerate(pairs):
                        nc.tensor.matmul(ps_col, lhsT=lw, rhs=rh,
                                         start=False, stop=(i == len(pairs) - 1),
                                         skip_group_check=True)

                # step 0: hp = 0
                rz0 = pg.tile([P, 8], F32, name="rz_s", bufs=2)
                nc.scalar.activation(rz0[:], xp_sb[:, 0:8, 0], AF.Sigmoid)
                t1 = pg.tile([P, DC], F32, name="t1_s", bufs=2)
                nc.vector.tensor_tensor(t1[:], rz0[:, 0:4], bhhn_sb[:], ALU.mult)
                nc.vector.tensor_tensor(t1[:], t1[:], xp_sb[:, 8:12, 0], ALU.add)
                n0 = pg.tile([P, DC], F32, name="n_s", bufs=2)
                nc.scalar.activation(n0[:], t1[:], AF.Tanh)
                # h0 = (1-z)*n = n - z*n
                tz = pg.tile([P, DC], F32, name="tz_s", bufs=2)
                nc.vector.tensor_tensor(tz[:], rz0[:, 4:8], n0[:], ALU.mult)
                nc.vector.tensor_tensor(h_sb[:, :, 0], n0[:], tz[:], ALU.subtract)
                emit_h_casts(0)

                for t in range(1, nsteps):
                    psB = pps.tile([P, DC], F32, name="psB")
                    psAr = pps.tile([P, DC], F32, name="psAr")
                    psAz = pps.tile([P, DC], F32, name="psAz")
                    # xp / b_hh_n preloads via one wide identity matmul each:
                    # independent of h[t-1], so they run under the previous
                    # step's tail (per-column interleaved groups are broken on
                    # HW; a single start=True covering the bank is correct).
                    # Order: psB then psA-r groups first so the serial gate
                    # chain (sigmoid_r -> ... -> h) starts mid-burst.
                    nc.tensor.matmul(psB[:, :], lhsT=ident_sb[:],
                                     rhs=bhhn_pre[:, :], start=True, stop=False)
                    nc.tensor.matmul(psAr[:, :], lhsT=ident_sb[:],
                                     rhs=xp_pre[:, 0:4, t], start=True, stop=False)
                    nc.tensor.matmul(psAz[:, :], lhsT=ident_sb[:],
                                     rhs=xp_pre[:, 4:8, t], start=True, stop=False)
                    for j in range(8, GC):
                        emit_gate_mms(psB[:, j - 8:j - 7], j, t)
                    for j in range(0, 4):
                        emit_gate_mms(psAr[:, j:j + 1], j, t)
                    for j in range(4, 8):
                        emit_gate_mms(psAz[:, j - 4:j - 3], j, t)
                    rr_ = pg.tile([P, DC], F32, name="r_s", bufs=2)
                    nc.scalar.activation(rr_[:], psAr[:], AF.Sigmoid)
                    zz_ = pg.tile([P, DC], F32, name="z_s", bufs=2)
                    nc.scalar.activation(zz_[:], psAz[:], AF.Sigmoid)
                    t1 = pg.tile([P, DC], F32, name="t1_s", bufs=2)
                    nc.vector.tensor_tensor(t1[:], rr_[:], psB[:], ALU.mult)
                    nc.vector.tensor_tensor(t1[:], t1[:], xp_sb[:, 8:12, t], ALU.add)
                    nn = pg.tile([P, DC], F32, name="n_s", bufs=2)
                    nc.scalar.activation(nn[:], t1[:], AF.Tanh)
                    # h = n + z*(h_prev - n); bf16 h16 written first (it alone
                    # gates the next step's matmuls), fp32 state off-path
                    tz = pg.tile([P, DC], F32, name="tz_s", bufs=2)
                    nc.vector.tensor_tensor(tz[:], h_sb[:, :, t - 1], nn[:], ALU.subtract)
                    nc.vector.tensor_tensor(tz[:], zz_[:], tz[:], ALU.mult)
                    if GRU_MODE == "f32":
                        nc.vector.tensor_tensor(h_sb[:, :, t], nn[:], tz[:], ALU.add)
                    else:
                        nc.vector.tensor_tensor(h16[:, :, t], nn[:], tz[:], ALU.add)
                        nc.vector.tensor_tensor(h_sb[:, :, t], nn[:], tz[:], ALU.add)
                        if GRU_MODE == "split":
                            lo = pg.tile([P, DC], F32, name="hlo_s", bufs=2)
                            nc.vector.tensor_tensor(lo[:], h_sb[:, :, t],
                                                    h16[:, :, t], ALU.subtract)
                            nc.scalar.activation(hlo16[:, :, t], lo[:], AF.Copy)

            # ---------------- phase 3: pair AllGather of h ----------------
            hT_dr = dram.tile([P, DC, N], F32)
            hpair_dr = dram.tile([2, P, DC, N], F32)
            nc.sync.dma_start(hT_dr[:], h_sb[:])
            nc.gpsimd.collective_compute(
                "AllGather", ALU.bypass, replica_groups=PAIRS,
                ins=[hT_dr.opt()], outs=[hpair_dr.opt()])

            # ---------------- phase 4: sampled, beta, gated scan ----------------
            with tc.tile_pool(name="ph4", bufs=1) as p4, \
                 tc.tile_pool(name="wch", bufs=4) as wch:
                aph_sb = p4.tile([P, DC, N], F32)
                suh_sb = p4.tile([P, DC, N], F32)
                nc.sync.dma_start(aph_sb[:], hpair_dr[0])
                nc.sync.dma_start(suh_sb[:], hpair_dr[1])
                noise_sb = p4.tile([P, DC, N], F32)
                nc.sync.dma_start(noise_sb[:],
                                  noiseT_d.rearrange("(k p) t -> p k t", p=P))
                bwrep_sb = p4.tile([P, DC, P], F32)
                nc.sync.dma_start(bwrep_sb[:],
                                  bwrep_d.rearrange("(k p) m -> p k m", p=P))

                beta_sb = p4.tile([P, N], F32)
                psb = ppb.tile([P, N], F32, name="ps_beta", tag="psbig")
                for k in range(DC):
                    nc.tensor.matmul(psb[:], lhsT=_r32(bwrep_sb[:, k, :]),
                                     rhs=_r32(suh_sb[:, k, :]),
                                     start=(k == 0), stop=(k == DC - 1))
                nc.scalar.activation(beta_sb[:], psb[:], AF.Sigmoid)
                forget_sb = p4.tile([P, N], F32)
                nc.scalar.activation(forget_sb[:], beta_sb[:], AF.Identity,
                                     bias=1.0, scale=-1.0)

                samp_sb = p4.tile([P, DC, N], F32)
                for k in range(DC):
                    psm = ppb.tile([P, N], F32, name="ps_mean", tag="psbig")
                    psv = ppb.tile([P, N], F32, name="ps_lv", tag="psbig")
                    for kk in range(DC):
                        mch = wch.tile([P, P], F32, name="aow_m")
                        nc.sync.dma_start(
                            mch[:], aowT_d[kk * P:(kk + 1) * P, k * P:(k + 1) * P])
                        nc.tensor.matmul(psm[:], lhsT=_r32(mch[:]),
                                         rhs=_r32(aph_sb[:, kk, :]),
                                         start=(kk == 0), stop=(kk == DC - 1))
                    for kk in range(DC):
                        vch = wch.tile([P, P], F32, name="aow_v")
                        nc.sync.dma_start(
                            vch[:], aowT_d[kk * P:(kk + 1) * P,
                                           (DC + k) * P:(DC + k + 1) * P])
                        nc.tensor.matmul(psv[:], lhsT=_r32(vch[:]),
                                         rhs=_r32(aph_sb[:, kk, :]),
                                         start=(kk == 0), stop=(kk == DC - 1))
                    std = p4.tile([P, N], F32, name="std_t", bufs=2)
                    nc.scalar.activation(std[:], psv[:], AF.Exp, scale=0.5)
                    nc.vector.tensor_tensor(std[:], noise_sb[:, k, :], std[:], ALU.mult)
                    nc.vector.tensor_tensor(samp_sb[:, k, :], psm[:], std[:], ALU.add)
                    # u = forget * sampled (in place), then scan
                    nc.vector.tensor_tensor(samp_sb[:, k, :], samp_sb[:, k, :],
                                            forget_sb[:], ALU.mult)
                    nc.vector.tensor_tensor_scan(gated_sb[:, k, :], beta_sb[:],
                                                 samp_sb[:, k, :], 0.0,
                                                 ALU.mult, ALU.add)

            # ---------------- phase 5: decoder ----------------
            y_dr = dram.tile([P, DC, N], F32)
            ysum_dr = dram.tile([P, DC, N], F32)
            with tc.tile_pool(name="ph5", bufs=1) as p5, \
                 tc.tile_pool(name="w2p", bufs=3) as w2p, \
                 tc.tile_pool(name="s2p", bufs=2) as s2p:
                db1_sb = p5.tile([P, HC], F32)
                nc.sync.dma_start(db1_sb[:], db1_d[:])
                hid_sb = p5.tile([P, HC, N], W2A_DT)
                with tc.tile_pool(name="dw1", bufs=4) as dw1p:
                    for m in range(HC):
                        ps = ppb.tile([P, N], F32, name="ps_hid", tag="psbig")
                        for k in range(DC):
                            wc = dw1p.tile([P, P], F32, name="dw1_c")
                            nc.sync.dma_start(
                                wc[:], dw1T_d[k * P:(k + 1) * P, m * P:(m + 1) * P])
                            nc.tensor.matmul(ps[:], lhsT=_r32(wc[:]),
                                             rhs=_r32(gated_sb[:, k, :]),
                                             start=(k == 0), stop=(k == DC - 1))
                        nc.scalar.activation(hid_sb[:, m, :], ps[:], AF.Silu,
                                             bias=db1_sb[:, m:m + 1])

                # s2 row vector [16, N] -> zero-padded to 128 partitions
                w2sT_sb = p5.tile([P, HC, R], W2A_DT)
                nc.sync.dma_start(w2sT_sb[:],
                                  w2sT_d.rearrange("(k p) r -> p k r", p=P))
                b2s_sb = p5.tile([R, 1], F32)
                nc.sync.dma_start(b2s_sb[:], b2s_d[:])
                s2big = p5.tile([P, N], W2A_DT)
                nc.vector.memset(s2big[:], 0.0)
                ps2 = ppb.tile([R, N], F32, name="ps_s2", tag="psbig")
                for kk in range(HC):
                    nc.tensor.matmul(ps2[:], lhsT=_r32(w2sT_sb[:, kk, :]),
                                     rhs=_r32(hid_sb[:, kk, :]),
                                     start=(kk == 0), stop=(kk == HC - 1))
                nc.scalar.activation(s2big[0:R, :], ps2[:], AF.Identity,
                                     bias=b2s_sb[:])

                sel_sb = p5.tile([P, RH * P], W2A_DT)
                nc.sync.dma_start(sel_sb[:], sel_d[:])
                b2a_sb = p5.tile([P, RH * DC], F32)
                nc.sync.dma_start(b2a_sb[:], b2a_d[:])

                y_sb = p5.tile([P, DC, N], F32)
                for rl in range(RH):
                    # broadcast s2[r] over 128 partitions via selector matmul
                    pbc = ppb.tile([P, N], F32, name="ps_bc", tag="psbig")
                    nc.tensor.matmul(pbc[:], lhsT=_r32(sel_sb[:, rl * P:(rl + 1) * P]),
                                     rhs=_r32(s2big[:]), start=True, stop=True)
                    s2bc = s2p.tile([P, N], F32, name="s2bc")
                    nc.vector.tensor_copy(out=s2bc[:], in_=pbc[:])
                    for db in range(DC):
                        cidx = rl * DC + db
                        w2c = w2p.tile([P, HC, P], W2A_DT, name="w2c")
                        nc.sync.dma_start(w2c[:], w2a_d[cidx])
                        pw = ppb.tile([P, N], F32, name="ps_w1", tag="psbig")
                        for kk in range(HC):
                            nc.tensor.matmul(pw[:], lhsT=_r32(w2c[:, kk, :]),
                                             rhs=_r32(hid_sb[:, kk, :]),
                                             start=(kk == 0), stop=(kk == HC - 1))
                        if rl == 0:
                            nc.vector.scalar_tensor_tensor(
                                y_sb[:, db, :], pw[:], b2a_sb[:, cidx:cidx + 1],
                                s2bc[:], ALU.add, ALU.mult)
                        else:
                            tmp = s2p.tile([P, N], F32, name="ytmp")
                            nc.vector.scalar_tensor_tensor(
                                tmp[:], pw[:], b2a_sb[:, cidx:cidx + 1],
                                s2bc[:], ALU.add, ALU.mult)
                            nc.vector.tensor_tensor(y_sb[:, db, :], y_sb[:, db, :],
                                                    tmp[:], ALU.add)

                # pairwise AllReduce of partial y
                nc.sync.dma_start(y_dr[:], y_sb[:])
                nc.gpsimd.collective_compute(
                    "AllReduce", ALU.add, replica_groups=PAIRS,
                    ins=[y_dr.opt()], outs=[ysum_dr.opt()])
                ysum_sb = p5.tile([P, DC, N], F32)
                nc.sync.dma_start(ysum_sb[:], ysum_dr[:])

                out_sb = p5.tile([P, DC, N], F32)
                for k in range(DC):
                    nc.vector.tensor_tensor(out_sb[:, k, :], gated_sb[:, k, :],
                                            ysum_sb[:, k, :], ALU.mult)
                    nc.vector.tensor_tensor(out_sb[:, k, :], out_sb[:, k, :],
                                            xT_sb[:, k, :], ALU.add)
                nc.sync.dma_start(outT_d[:], out_sb[:])

    nc.compile()
    return nc


_PROG = {}


def _get_program(nsteps=N):
    if nsteps not in _PROG:
        _PROG[nsteps] = _build_program(nsteps)
    return _PROG[nsteps]


def _prep_in_maps(inputs):
    f = np.float32
    res = np.asarray(inputs["residual_stream"], f)
    noi = np.asarray(inputs["noise"], f)
    gru_w = {
        0: (np.asarray(inputs["ap_w_ih"], f), np.asarray(inputs["ap_w_hh"], f),
            np.asarray(inputs["ap_b_ih"], f), np.asarray(inputs["ap_b_hh"], f)),
        1: (np.asarray(inputs["su_w_ih"], f), np.asarray(inputs["su_w_hh"], f),
            np.asarray(inputs["su_b_ih"], f), np.asarray(inputs["su_b_hh"], f)),
    }
    aowT = np.ascontiguousarray(np.asarray(inputs["ap_out_w"], f).T)      # [D, 2D]
    bwrep = np.ascontiguousarray(
        np.tile(np.asarray(inputs["beta_w"], f).reshape(D, 1), (1, P)))   # [D, P]
    dw1T = np.ascontiguousarray(np.asarray(inputs["dec_w1"], f).T)        # [D, H]
    db1 = np.ascontiguousarray(
        np.asarray(inputs["dec_b1"], f).reshape(HC, P).T)                 # [P, HC]
    w2 = np.asarray(inputs["dec_w2"], f)                                  # [2DR, H]
    b2 = np.asarray(inputs["dec_b2"], f)                                  # [2DR]
    W2a = w2[:D * R].reshape(D, R, H)                                     # [d, r, h]
    B2a = b2[:D * R].reshape(D, R)
    W2s = w2[D * R:].reshape(D, R, H).sum(axis=0)                         # [R, H]
    b2s = b2[D * R:].reshape(D, R).sum(axis=0).reshape(R, 1)              # [R, 1]
    w2sT = np.ascontiguousarray(W2s.T)                                    # [H, R]

    np_w2a_dt = np.float32 if W2A_DT == F32 else np.dtype("bfloat16")

    in_maps = []
    for c in range(NCORES):
        b, role = c // 2, c % 2
        w_ih, w_hh, b_ih, b_hh = gru_w[role]
        xbias = b_ih + np.concatenate([b_hh[:2 * D], np.zeros(D, f)])
        xbias = np.ascontiguousarray(xbias.reshape(GC, P).T)              # [P, GC]
        bhhn = np.ascontiguousarray(b_hh[2 * D:].reshape(DC, P).T)        # [P, DC]

        # rank-half shard of the w1-part of dec_w2
        rsl = slice(role * RH, (role + 1) * RH)
        sub = W2a[:, rsl, :]                                              # [D, RH, H]
        t = sub.transpose(1, 0, 2).reshape(RH, DC, P, H)                  # [rl,db,m,h]
        w2a_tiled = np.ascontiguousarray(
            t.transpose(0, 1, 3, 2).reshape(RH * DC, HC, P, P)
            .transpose(0, 2, 1, 3))                                       # [cidx,p,kk,m]
        b2a_c = np.zeros((P, RH * DC), f)
        for rl in range(RH):
            for db in range(DC):
                b2a_c[:, rl * DC + db] = B2a[db * P:(db + 1) * P, role * RH + rl]
        sel = np.zeros((P, RH * P), f)
        for rl in range(RH):
            sel[role * RH + rl, rl * P:(rl + 1) * P] = 1.0

        whhT = np.ascontiguousarray(w_hh.T)
        im = {
            "xT": np.ascontiguousarray(res[b].T),
            "noiseT": np.ascontiguousarray(noi[b].T),
            "wihT": np.ascontiguousarray(w_ih.T),
            "whhT": whhT.astype(
                np.float32 if GRU_MODE == "f32" else np.dtype("bfloat16")),
            "xbias": xbias,
            "bhhn": bhhn,
            "aowT": aowT,
            "bwrep": bwrep,
            "dw1T": dw1T,
            "db1": db1,
            "w2a": w2a_tiled.astype(np_w2a_dt),
            "b2a": b2a_c,
            "w2sT": w2sT.astype(np_w2a_dt),
            "b2s": b2s,
            "sel": sel.astype(np_w2a_dt),
            "ident": np.eye(P, dtype=np.float32).astype(
                np.float32 if GRU_MODE == "f32" else np.dtype("bfloat16")),
        }
        if GRU_MODE == "split":
            hi = whhT.astype(np.dtype("bfloat16"))
            im["whhLo"] = (whhT - hi.astype(np.float32)).astype(np.dtype("bfloat16"))
        in_maps.append(im)
    return in_maps


def kernel(**inputs):
    nc = _get_program()
    in_maps = _prep_in_maps(inputs)
    rr = run_bass_kernel_spmd(nc, in_maps, list(range(NCORES)))
    modified = np.empty((B, N, D), np.float32)
    for b in range(B):
        o = rr.results[2 * b]["outT"]                      # [P, DC, N]
        modified[b] = o.transpose(2, 1, 0).reshape(N, D)
    return modified, np.zeros((), np.float32)


def _install_ntff_shim():
    """The image's antenv lacks axon_hooks; synthesize it and register the
    ctypes-based NTFF profile hook from trn_agent_boot."""
    import types
    if "antenv.axon_hooks" in sys.modules:
        return
    mod = types.ModuleType("antenv.axon_hooks")
    holder = {}
    mod.set_axon_ntff_profile_hook = lambda h: holder.__setitem__("h", h)
    mod.get_axon_ntff_profile_hook = lambda: holder.get("h")
    sys.modules["antenv.axon_hooks"] = mod
    import antenv
    antenv.axon_hooks = mod
    from trn_agent_boot.trn_boot import _ntff_profile_via_ctypes
    mod.set_axon_ntff_profile_hook(
        _ntff_profile_via_ctypes("/opt/axon/libaxon_pjrt.so"))


def profile_once(inputs, trace_kwargs=None, tmpdir=None):
    """Run once with NTFF tracing; returns BassKernelResults with
    exec_time_ns / trace. NTFF + trace artifacts land in tmpdir."""
    import tempfile
    import concourse.bass_utils as bu
    _install_ntff_shim()
    bu.upload_artifacts = lambda d: str(d)  # no bucket in this container
    nc = _get_program()
    in_maps = _prep_in_maps(inputs)
    if tmpdir is None:
        tmpdir = tempfile.mkdtemp(prefix="ntff_")
    rr = run_bass_kernel_spmd(nc, in_maps, list(range(NCORES)), trace=True,
                              tmpdir=tmpdir, trace_kwargs=trace_kwargs or {})
    return rr


if __name__ == "__main__":
    import time
    nsteps = int(sys.argv[1]) if len(sys.argv) > 1 else N
    t0 = time.time()
    nc = _build_program(nsteps)
    print(f"build+compile nsteps={nsteps}:", time.time() - t0)


# revision 16
# speedup vs baseline: 6.5996x; 1.1131x over previous
"""Trainium2 Bass kernel for nn_MetaController (GRU meta-controller).

Architecture (B=4, N=512, D=512, H=1024, R=16):
  - 2 GRUs (action-proposer, switching-unit) over N=512 sequential steps
  - reparameterized sampling, sigmoid beta gate
  - gated linear scan over time (tensor_tensor_scan)
  - decoder MLP -> low-rank hypernetwork; algebraic simplifications:
      * w2-half of dec_w2 only appears as sum over d -> pre-reduced on host to [16,H]
      * y[d] = sum_r w1[d,r] * s2[r] computed via r-major GEMM + DVE contraction

Sharding (8 cores, identical SPMD program, per-core *data* differs):
  core c: batch b=c//2, role=c%2 (0: ap-GRU + r-half 0, 1: su-GRU + r-half 1)
  - each core runs one GRU chain (B=1) -- the recurrence is LS-bandwidth-bound,
    so B=1 per core costs the same as B=4 and uses all 8 cores
  - pairwise AllGather exchanges the two GRU outputs within a (batch) pair
  - both pair cores compute sampled/beta/scan for the full 512 tokens
  - decoder W2a GEMM sharded by rank-half (8 of 16 r per core), partial y
    summed with a pairwise AllReduce; even cores' output is used by the host
"""

import sys

sys.path.insert(0, "/opt/trn_rl_repo")

import numpy as np

import concourse.bass as bass
import concourse.tile as tile
from concourse import bacc, mybir
from concourse.bass_utils import run_bass_kernel_spmd

F32 = mybir.dt.float32
F32R = mybir.dt.float32r
BF16 = mybir.dt.bfloat16
AF = mybir.ActivationFunctionType
ALU = mybir.AluOpType

B, N, D = 4, 512, 512
G = 3 * D            # 1536 gate width
H = 1024             # decoder hidden
R = 16               # low rank
P = 128
DC = D // P          # 4 d-chunks
GC = G // P          # 12 gate chunks
HC = H // P          # 8 hidden chunks
RH = R // 2          # 8 ranks per core
NCORES = 8
PAIRS = [[2 * i, 2 * i + 1] for i in range(4)]

# precision knobs
GRU_MODE = "bf16"    # "f32" | "bf16" | "split" (hi/lo bf16, ~fp32 accuracy)
BIG_F32R = False     # float32r GEMMs: verifier requires f32r-typed producers; off
W2A_DT = F32         # dtype of the big decoder GEMM (lhsT + rhs)

GRU_DT = F32 if GRU_MODE == "f32" else BF16


def _r32(ap):
    """View an fp32 AP as float32r for full-rate PE streaming."""
    return ap.bitcast(mybir.dt.float32r) if BIG_F32R else ap


def _build_program(nsteps=N):
    nc = bacc.Bacc("TRN2", target_bir_lowering=False, debug=False,
                   num_devices=NCORES)

    def din(name, shape, dt=F32):
        return nc.dram_tensor(name, list(shape), dt, kind="ExternalInput").ap()

    xT_d = din("xT", [D, N], F32R)                    # residual[b].T
    noiseT_d = din("noiseT", [D, N])
    wihT_d = din("wihT", [D, G], F32R)                # this core's GRU W_ih^T
    whhT_d = din("whhT", [D, G], GRU_DT)        # W_hh^T (LS-streamed)
    if GRU_MODE == "split":
        whhlo_d = din("whhLo", [D, G], BF16)    # W_hh^T residual (hi/lo split)
    xbias_d = din("xbias", [P, GC])             # b_ih (+b_hh for r,z) chunk-major
    bhhn_d = din("bhhn", [P, DC])               # b_hh n-part
    aowT_d = din("aowT", [D, 2 * D], F32R)            # ap_out_w^T
    bwrep_d = din("bwrep", [D, P], F32R)              # beta_w^T replicated to 128 cols
    dw1T_d = din("dw1T", [D, H], F32R)                # dec_w1^T
    db1_d = din("db1", [P, HC])
    w2a_d = din("w2a", [RH * DC, P, HC, P], F32R)  # pre-tiled lhsT chunks
    b2a_d = din("b2a", [P, RH * DC])
    w2sT_d = din("w2sT", [H, R], F32R)                # pre-reduced w2-half
    b2s_d = din("b2s", [R, 1])
    sel_d = din("sel", [R, RH * P], F32R)             # padded row-selectors for r bcast
    ident_d = din("ident", [P, P], GRU_DT)      # identity for psum xp preload

    outT_d = nc.dram_tensor("outT", [P, DC, N], F32, kind="ExternalOutput").ap()

    with tile.TileContext(nc) as tc:
        from contextlib import ExitStack
        with ExitStack() as ctx:
            perm = ctx.enter_context(tc.tile_pool(name="perm", bufs=1))
            ppb = ctx.enter_context(tc.tile_pool(name="ppb", bufs=2, space="PSUM"))
            pps = ctx.enter_context(tc.tile_pool(name="pps", bufs=2, space="PSUM"))
            dram = ctx.enter_context(tc.tile_pool(name="dram", bufs=1, space="DRAM"))

            xT_sb = perm.tile([P, DC, N], F32R)
            nc.sync.dma_start(xT_sb[:], xT_d.rearrange("(k p) t -> p k t", p=P))
            h_sb = perm.tile([P, DC, N], F32R)
            gated_sb = perm.tile([P, DC, N], F32R)
            xbias_sb = perm.tile([P, GC], F32)
            nc.sync.dma_start(xbias_sb[:], xbias_d[:])
            bhhn_sb = perm.tile([P, DC], F32)
            nc.sync.dma_start(bhhn_sb[:], bhhn_d[:])

            # ---------------- phase 1+2: xp GEMM, GRU recurrence ----------------
            with tc.tile_pool(name="gru", bufs=1) as pg:
                whh_sb = pg.tile([P, DC, G], GRU_DT)
                nc.sync.dma_start(whh_sb[:], whhT_d.rearrange("(k p) g -> p k g", p=P))
                if GRU_MODE == "split":
                    whhlo_sb = pg.tile([P, DC, G], BF16)
                    nc.sync.dma_start(whhlo_sb[:],
                                      whhlo_d.rearrange("(k p) g -> p k g", p=P))
                xp_sb = pg.tile([P, GC, N], F32)

                with tc.tile_pool(name="ph1", bufs=1) as p1:
                    wih_sb = p1.tile([P, DC, G], F32R)
                    nc.sync.dma_start(wih_sb[:],
                                      wihT_d.rearrange("(k p) g -> p k g", p=P))
                    for m in range(GC):
                        ps = ppb.tile([P, N], F32, name="ps_xp", tag="psbig")
                        for k in range(DC):
                            nc.tensor.matmul(ps[:],
                                             lhsT=_r32(wih_sb[:, k, m * P:(m + 1) * P]),
                                             rhs=_r32(xT_sb[:, k, :]),
                                             start=(k == 0), stop=(k == DC - 1))
                        nc.scalar.activation(xp_sb[:, m, :], ps[:], AF.Identity,
                                             bias=xbias_sb[:, m:m + 1])

                # ---- recurrence ----
                ident_sb = pg.tile([P, P], GRU_DT)
                nc.sync.dma_start(ident_sb[:], ident_d[:])
                if GRU_MODE != "f32":
                    h16 = pg.tile([P, DC, N], BF16)
                    # bf16 copies of xp (r,z parts) and bhhn for the psum
                    # preload matmuls (identity lhsT, run in the tail shadow)
                    xp16 = pg.tile([P, 8, N], BF16)
                    for m in range(8):
                        nc.scalar.activation(xp16[:, m, :], xp_sb[:, m, :], AF.Copy)
                    bhhn16 = pg.tile([P, DC], BF16)
                    nc.scalar.activation(bhhn16[:], bhhn_sb[:], AF.Copy)
                    xp_pre, bhhn_pre = xp16, bhhn16
                else:
                    xp_pre, bhhn_pre = xp_sb, bhhn_sb
                if GRU_MODE == "split":
                    hlo16 = pg.tile([P, DC, N], BF16)

                def emit_h_casts(t):
                    """After h_sb[:, :, t] is written, produce the bf16 views."""
                    if GRU_MODE == "f32":
                        return
                    nc.scalar.activation(h16[:, :, t], h_sb[:, :, t].bitcast(F32), AF.Copy)
                    if GRU_MODE == "split":
                        lo = pg.tile([P, DC], F32, name="hlo_s", bufs=2)
                        nc.vector.tensor_tensor(lo[:], h_sb[:, :, t].bitcast(F32), h16[:, :, t],
                                                ALU.subtract)
                        nc.scalar.activation(hlo16[:, :, t], lo[:], AF.Copy)

                def emit_gate_mms(ps_col, j, t):
                    """Accumulate hp for gate chunk j at step t into psum col."""
                    gsl = slice(j * P, (j + 1) * P)
                    pairs = []
                    for k in range(DC):
                        if GRU_MODE == "f32":
                            pairs.append((whh_sb[:, k, gsl], h_sb[:, k, t - 1:t]))
                        else:
                            pairs.append((whh_sb[:, k, gsl], h16[:, k, t - 1:t]))
                            if GRU_MODE == "split":
                                pairs.append((whh_sb[:, k, gsl],
                                              hlo16[:, k, t - 1:t]))
                                pairs.append((whhlo_sb[:, k, gsl],
                                              h16[:, k, t - 1:t]))
                    for i, (lw, rh) in enumerate(pairs):
                        nc.tensor.matmul(ps_col, lhsT=lw, rhs=rh,
                                         start=False, stop=(i == len(pairs) - 1),
                                         skip_group_check=True)

                # step 0: hp = 0
                rz0 = pg.tile([P, 8], F32, name="rz_s", bufs=2)
                nc.scalar.activation(rz0[:], xp_sb[:, 0:8, 0], AF.Sigmoid)
                t1 = pg.tile([P, DC], F32, name="t1_s", bufs=2)
                nc.vector.tensor_tensor(t1[:], rz0[:, 0:4], bhhn_sb[:], ALU.mult)
                nc.vector.tensor_tensor(t1[:], t1[:], xp_sb[:, 8:12, 0], ALU.add)
                n0 = pg.tile([P, DC], F32, name="n_s", bufs=2)
                nc.scalar.activation(n0[:], t1[:], AF.Tanh)
                # h0 = (1-z)*n = n - z*n
                tz = pg.tile([P, DC], F32, name="tz_s", bufs=2)
                nc.vector.tensor_tensor(tz[:], rz0[:, 4:8], n0[:], ALU.mult)
                nc.vector.tensor_tensor(h_sb[:, :, 0], n0[:], tz[:], ALU.subtract)
                emit_h_casts(0)

                for t in range(1, nsteps):
                    psB = pps.tile([P, DC], F32, name="psB")
                    psAr = pps.tile([P, DC], F32, name="psAr")
                    psAz = pps.tile([P, DC], F32, name="psAz")
                    # xp / b_hh_n preloads via one wide identity matmul each:
                    # independent of h[t-1], so they run under the previous
                    # step's tail (per-column interleaved groups are broken on
                    # HW; a single start=True covering the bank is correct).
                    # Order: psB then psA-r groups first so the serial gate
                    # chain (sigmoid_r -> ... -> h) starts mid-burst.
                    nc.tensor.matmul(psB[:, :], lhsT=ident_sb[:],
                                     rhs=bhhn_pre[:, :], start=True, stop=False)
                    nc.tensor.matmul(psAr[:, :], lhsT=ident_sb[:],
                                     rhs=xp_pre[:, 0:4, t], start=True, stop=False)
                    nc.tensor.matmul(psAz[:, :], lhsT=ident_sb[:],
                                     rhs=xp_pre[:, 4:8, t], start=True, stop=False)
                    for j in range(8, GC):
                        emit_gate_mms(psB[:, j - 8:j - 7], j, t)
                    for j in range(0, 4):
                        emit_gate_mms(psAr[:, j:j + 1], j, t)
                    for j in range(4, 8):
                        emit_gate_mms(psAz[:, j - 4:j - 3], j, t)
                    rr_ = pg.tile([P, DC], F32, name="r_s", bufs=2)
                    nc.scalar.activation(rr_[:], psAr[:], AF.Sigmoid)
                    zz_ = pg.tile([P, DC], F32, name="z_s", bufs=2)
                    nc.scalar.activation(zz_[:], psAz[:], AF.Sigmoid)
                    t1 = pg.tile([P, DC], F32, name="t1_s", bufs=2)
                    nc.vector.tensor_tensor(t1[:], rr_[:], psB[:], ALU.mult)
                    nc.vector.tensor_tensor(t1[:], t1[:], xp_sb[:, 8:12, t], ALU.add)
                    nn = pg.tile([P, DC], F32, name="n_s", bufs=2)
                    nc.scalar.activation(nn[:], t1[:], AF.Tanh)
                    # h = n + z*(h_prev - n); bf16 h16 written first (it alone
                    # gates the next step's matmuls), fp32 state off-path
                    tz = pg.tile([P, DC], F32, name="tz_s", bufs=2)
                    nc.vector.tensor_tensor(tz[:], h_sb[:, :, t - 1].bitcast(F32), nn[:], ALU.subtract)
                    nc.vector.tensor_tensor(tz[:], zz_[:], tz[:], ALU.mult)
                    if GRU_MODE == "f32":
                        nc.vector.tensor_tensor(h_sb[:, :, t], nn[:], tz[:], ALU.add)
                    else:
                        nc.vector.tensor_tensor(h16[:, :, t], nn[:], tz[:], ALU.add)
                        nc.vector.tensor_tensor(h_sb[:, :, t], nn[:], tz[:], ALU.add)
                        if GRU_MODE == "split":
                            lo = pg.tile([P, DC], F32, name="hlo_s", bufs=2)
                            nc.vector.tensor_tensor(lo[:], h_sb[:, :, t].bitcast(F32),
                                                    h16[:, :, t], ALU.subtract)
                            nc.scalar.activation(hlo16[:, :, t], lo[:], AF.Copy)

            # ---------------- phase 3: pair AllGather of h ----------------
            hT_dr = dram.tile([P, DC, N], F32R)
            hpair_dr = dram.tile([2, P, DC, N], F32R)
            nc.sync.dma_start(hT_dr[:], h_sb[:])
            nc.gpsimd.collective_compute(
                "AllGather", ALU.bypass, replica_groups=PAIRS,
                ins=[hT_dr.opt()], outs=[hpair_dr.opt()])

            # ---------------- phase 4: sampled, beta, gated scan ----------------
            with tc.tile_pool(name="ph4", bufs=1) as p4, \
                 tc.tile_pool(name="wch", bufs=4) as wch:
                aph_sb = p4.tile([P, DC, N], F32R)
                suh_sb = p4.tile([P, DC, N], F32R)
                nc.sync.dma_start(aph_sb[:], hpair_dr[0])
                nc.sync.dma_start(suh_sb[:], hpair_dr[1])
                noise_sb = p4.tile([P, DC, N], F32)
                nc.sync.dma_start(noise_sb[:],
                                  noiseT_d.rearrange("(k p) t -> p k t", p=P))
                bwrep_sb = p4.tile([P, DC, P], F32R)
                nc.sync.dma_start(bwrep_sb[:],
                                  bwrep_d.rearrange("(k p) m -> p k m", p=P))

                beta_sb = p4.tile([P, N], F32)
                psb = ppb.tile([P, N], F32, name="ps_beta", tag="psbig")
                for k in range(DC):
                    nc.tensor.matmul(psb[:], lhsT=_r32(bwrep_sb[:, k, :]),
                                     rhs=_r32(suh_sb[:, k, :]),
                                     start=(k == 0), stop=(k == DC - 1))
                nc.scalar.activation(beta_sb[:], psb[:], AF.Sigmoid)
                forget_sb = p4.tile([P, N], F32)
                nc.scalar.activation(forget_sb[:], beta_sb[:], AF.Identity,
                                     bias=1.0, scale=-1.0)

                samp_sb = p4.tile([P, DC, N], F32)
                for k in range(DC):
                    psm = ppb.tile([P, N], F32, name="ps_mean", tag="psbig")
                    psv = ppb.tile([P, N], F32, name="ps_lv", tag="psbig")
                    for kk in range(DC):
                        mch = wch.tile([P, P], F32R, name="aow_m")
                        nc.sync.dma_start(
                            mch[:], aowT_d[kk * P:(kk + 1) * P, k * P:(k + 1) * P])
                        nc.tensor.matmul(psm[:], lhsT=_r32(mch[:]),
                                         rhs=_r32(aph_sb[:, kk, :]),
                                         start=(kk == 0), stop=(kk == DC - 1))
                    for kk in range(DC):
                        vch = wch.tile([P, P], F32R, name="aow_v")
                        nc.sync.dma_start(
                            vch[:], aowT_d[kk * P:(kk + 1) * P,
                                           (DC + k) * P:(DC + k + 1) * P])
                        nc.tensor.matmul(psv[:], lhsT=_r32(vch[:]),
                                         rhs=_r32(aph_sb[:, kk, :]),
                                         start=(kk == 0), stop=(kk == DC - 1))
                    std = p4.tile([P, N], F32, name="std_t", bufs=2)
                    nc.scalar.activation(std[:], psv[:], AF.Exp, scale=0.5)
                    nc.vector.tensor_tensor(std[:], noise_sb[:, k, :], std[:], ALU.mult)
                    nc.vector.tensor_tensor(samp_sb[:, k, :], psm[:], std[:], ALU.add)
                    # u = forget * sampled (in place), then scan
                    nc.vector.tensor_tensor(samp_sb[:, k, :], samp_sb[:, k, :],
                                            forget_sb[:], ALU.mult)
                    nc.vector.tensor_tensor_scan(gated_sb[:, k, :], beta_sb[:],
                                                 samp_sb[:, k, :], 0.0,
                                                 ALU.mult, ALU.add)

            # ---------------- phase 5: decoder ----------------
            y_dr = dram.tile([P, DC, N], F32)
            ysum_dr = dram.tile([P, DC, N], F32)
            with tc.tile_pool(name="ph5", bufs=1) as p5, \
                 tc.tile_pool(name="w2p", bufs=3) as w2p, \
                 tc.tile_pool(name="s2p", bufs=2) as s2p:
                db1_sb = p5.tile([P, HC], F32)
                nc.sync.dma_start(db1_sb[:], db1_d[:])
                hid_sb = p5.tile([P, HC, N], F32R)
                with tc.tile_pool(name="dw1", bufs=4) as dw1p:
                    for m in range(HC):
                        ps = ppb.tile([P, N], F32, name="ps_hid", tag="psbig")
                        for k in range(DC):
                            wc = dw1p.tile([P, P], F32R, name="dw1_c")
                            nc.sync.dma_start(
                                wc[:], dw1T_d[k * P:(k + 1) * P, m * P:(m + 1) * P])
                            nc.tensor.matmul(ps[:], lhsT=_r32(wc[:]),
                                             rhs=_r32(gated_sb[:, k, :]),
                                             start=(k == 0), stop=(k == DC - 1))
                        nc.scalar.activation(hid_sb[:, m, :], ps[:], AF.Silu,
                                             bias=db1_sb[:, m:m + 1])

                # s2 row vector [16, N] -> zero-padded to 128 partitions
                w2sT_sb = p5.tile([P, HC, R], F32R)
                nc.sync.dma_start(w2sT_sb[:],
                                  w2sT_d.rearrange("(k p) r -> p k r", p=P))
                b2s_sb = p5.tile([R, 1], F32)
                nc.sync.dma_start(b2s_sb[:], b2s_d[:])
                s2big = p5.tile([R, N], F32R)
                ps2 = ppb.tile([R, N], F32, name="ps_s2", tag="psbig")
                for kk in range(HC):
                    nc.tensor.matmul(ps2[:], lhsT=_r32(w2sT_sb[:, kk, :]),
                                     rhs=_r32(hid_sb[:, kk, :]),
                                     start=(kk == 0), stop=(kk == HC - 1))
                nc.scalar.activation(s2big[:], ps2[:], AF.Identity,
                                     bias=b2s_sb[:])

                sel_sb = p5.tile([R, RH * P], F32R)
                nc.sync.dma_start(sel_sb[:], sel_d[:])
                b2a_sb = p5.tile([P, RH * DC], F32)
                nc.sync.dma_start(b2a_sb[:], b2a_d[:])

                y_sb = p5.tile([P, DC, N], F32)
                for rl in range(RH):
                    # broadcast s2[r] over 128 partitions via selector matmul
                    pbc = ppb.tile([P, N], F32, name="ps_bc", tag="psbig")
                    nc.tensor.matmul(pbc[:], lhsT=_r32(sel_sb[:, rl * P:(rl + 1) * P]),
                                     rhs=_r32(s2big[:]), start=True, stop=True)
                    s2bc = s2p.tile([P, N], F32, name="s2bc")
                    nc.vector.tensor_copy(out=s2bc[:], in_=pbc[:])
                    for db in range(DC):
                        cidx = rl * DC + db
                        w2c = w2p.tile([P, HC, P], F32R, name="w2c")
                        nc.sync.dma_start(w2c[:], w2a_d[cidx])
                        pw = ppb.tile([P, N], F32, name="ps_w1", tag="psbig")
                        for kk in range(HC):
                            nc.tensor.matmul(pw[:], lhsT=_r32(w2c[:, kk, :]),
                                             rhs=_r32(hid_sb[:, kk, :]),
                                             start=(kk == 0), stop=(kk == HC - 1))
                        if rl == 0:
                            nc.vector.scalar_tensor_tensor(
                                y_sb[:, db, :], pw[:], b2a_sb[:, cidx:cidx + 1],
                                s2bc[:], ALU.add, ALU.mult)
                        else:
                            tmp = s2p.tile([P, N], F32, name="ytmp")
                            nc.vector.scalar_tensor_tensor(
                                tmp[:], pw[:], b2a_sb[:, cidx:cidx + 1],
                                s2bc[:], ALU.add, ALU.mult)
                            nc.vector.tensor_tensor(y_sb[:, db, :], y_sb[:, db, :],
                                                    tmp[:], ALU.add)

                # pairwise AllReduce of partial y
                nc.sync.dma_start(y_dr[:], y_sb[:])
                nc.gpsimd.collective_compute(
                    "AllReduce", ALU.add, replica_groups=PAIRS,
                    ins=[y_dr.opt()], outs=[ysum_dr.opt()])
                ysum_sb = p5.tile([P, DC, N], F32)
                nc.sync.dma_start(ysum_sb[:], ysum_dr[:])

                out_sb = p5.tile([P, DC, N], F32)
                for k in range(DC):
                    nc.vector.tensor_tensor(out_sb[:, k, :], gated_sb[:, k, :].bitcast(F32),
                                            ysum_sb[:, k, :], ALU.mult)
                    nc.vector.tensor_tensor(out_sb[:, k, :], out_sb[:, k, :],
                                            xT_sb[:, k, :].bitcast(F32), ALU.add)
                nc.sync.dma_start(outT_d[:], out_sb[:])

    nc.compile()
    return nc


_PROG = {}


def _get_program(nsteps=N):
    if nsteps not in _PROG:
        _PROG[nsteps] = _build_program(nsteps)
    return _PROG[nsteps]


def _prep_in_maps(inputs):
    f = np.float32
    res = np.asarray(inputs["residual_stream"], f)
    noi = np.asarray(inputs["noise"], f)
    gru_w = {
        0: (np.asarray(inputs["ap_w_ih"], f), np.asarray(inputs["ap_w_hh"], f),
            np.asarray(inputs["ap_b_ih"], f), np.asarray(inputs["ap_b_hh"], f)),
        1: (np.asarray(inputs["su_w_ih"], f), np.asarray(inputs["su_w_hh"], f),
            np.asarray(inputs["su_b_ih"], f), np.asarray(inputs["su_b_hh"], f)),
    }
    aowT = np.ascontiguousarray(np.asarray(inputs["ap_out_w"], f).T)      # [D, 2D]
    bwrep = np.ascontiguousarray(
        np.tile(np.asarray(inputs["beta_w"], f).reshape(D, 1), (1, P)))   # [D, P]
    dw1T = np.ascontiguousarray(np.asarray(inputs["dec_w1"], f).T)        # [D, H]
    db1 = np.ascontiguousarray(
        np.asarray(inputs["dec_b1"], f).reshape(HC, P).T)                 # [P, HC]
    w2 = np.asarray(inputs["dec_w2"], f)                                  # [2DR, H]
    b2 = np.asarray(inputs["dec_b2"], f)                                  # [2DR]
    W2a = w2[:D * R].reshape(D, R, H)                                     # [d, r, h]
    B2a = b2[:D * R].reshape(D, R)
    W2s = w2[D * R:].reshape(D, R, H).sum(axis=0)                         # [R, H]
    b2s = b2[D * R:].reshape(D, R).sum(axis=0).reshape(R, 1)              # [R, 1]
    w2sT = np.ascontiguousarray(W2s.T)                                    # [H, R]

    np_w2a_dt = np.float32 if W2A_DT == F32 else np.dtype("bfloat16")

    in_maps = []
    for c in range(NCORES):
        b, role = c // 2, c % 2
        w_ih, w_hh, b_ih, b_hh = gru_w[role]
        xbias = b_ih + np.concatenate([b_hh[:2 * D], np.zeros(D, f)])
        xbias = np.ascontiguousarray(xbias.reshape(GC, P).T)              # [P, GC]
        bhhn = np.ascontiguousarray(b_hh[2 * D:].reshape(DC, P).T)        # [P, DC]

        # rank-half shard of the w1-part of dec_w2
        rsl = slice(role * RH, (role + 1) * RH)
        sub = W2a[:, rsl, :]                                              # [D, RH, H]
        t = sub.transpose(1, 0, 2).reshape(RH, DC, P, H)                  # [rl,db,m,h]
        w2a_tiled = np.ascontiguousarray(
            t.transpose(0, 1, 3, 2).reshape(RH * DC, HC, P, P)
            .transpose(0, 2, 1, 3))                                       # [cidx,p,kk,m]
        b2a_c = np.zeros((P, RH * DC), f)
        for rl in range(RH):
            for db in range(DC):
                b2a_c[:, rl * DC + db] = B2a[db * P:(db + 1) * P, role * RH + rl]
        sel = np.zeros((R, RH * P), f)
        for rl in range(RH):
            sel[role * RH + rl, rl * P:(rl + 1) * P] = 1.0

        whhT = np.ascontiguousarray(w_hh.T)
        im = {
            "xT": np.ascontiguousarray(res[b].T),
            "noiseT": np.ascontiguousarray(noi[b].T),
            "wihT": np.ascontiguousarray(w_ih.T),
            "whhT": whhT.astype(
                np.float32 if GRU_MODE == "f32" else np.dtype("bfloat16")),
            "xbias": xbias,
            "bhhn": bhhn,
            "aowT": aowT,
            "bwrep": bwrep,
            "dw1T": dw1T,
            "db1": db1,
            "w2a": w2a_tiled.astype(np_w2a_dt),
            "b2a": b2a_c,
            "w2sT": w2sT.astype(np_w2a_dt),
            "b2s": b2s,
            "sel": sel.astype(np_w2a_dt),
            "ident": np.eye(P, dtype=np.float32).astype(
                np.float32 if GRU_MODE == "f32" else np.dtype("bfloat16")),
        }
        if GRU_MODE == "split":
            hi = whhT.astype(np.dtype("bfloat16"))
            im["whhLo"] = (whhT - hi.astype(np.float32)).astype(np.dtype("bfloat16"))
        in_maps.append(im)
    return in_maps


def kernel(**inputs):
    nc = _get_program()
    in_maps = _prep_in_maps(inputs)
    rr = run_bass_kernel_spmd(nc, in_maps, list(range(NCORES)))
    modified = np.empty((B, N, D), np.float32)
    for b in range(B):
        o = rr.results[2 * b]["outT"]                      # [P, DC, N]
        modified[b] = o.transpose(2, 1, 0).reshape(N, D)
    return modified, np.zeros((), np.float32)


def _install_ntff_shim():
    """The image's antenv lacks axon_hooks; synthesize it and register the
    ctypes-based NTFF profile hook from trn_agent_boot."""
    import types
    if "antenv.axon_hooks" in sys.modules:
        return
    mod = types.ModuleType("antenv.axon_hooks")
    holder = {}
    mod.set_axon_ntff_profile_hook = lambda h: holder.__setitem__("h", h)
    mod.get_axon_ntff_profile_hook = lambda: holder.get("h")
    sys.modules["antenv.axon_hooks"] = mod
    import antenv
    antenv.axon_hooks = mod
    from trn_agent_boot.trn_boot import _ntff_profile_via_ctypes
    mod.set_axon_ntff_profile_hook(
        _ntff_profile_via_ctypes("/opt/axon/libaxon_pjrt.so"))


def profile_once(inputs, trace_kwargs=None, tmpdir=None):
    """Run once with NTFF tracing; returns BassKernelResults with
    exec_time_ns / trace. NTFF + trace artifacts land in tmpdir."""
    import tempfile
    import concourse.bass_utils as bu
    _install_ntff_shim()
    bu.upload_artifacts = lambda d: str(d)  # no bucket in this container
    nc = _get_program()
    in_maps = _prep_in_maps(inputs)
    if tmpdir is None:
        tmpdir = tempfile.mkdtemp(prefix="ntff_")
    rr = run_bass_kernel_spmd(nc, in_maps, list(range(NCORES)), trace=True,
                              tmpdir=tmpdir, trace_kwargs=trace_kwargs or {})
    return rr


if __name__ == "__main__":
    import time
    nsteps = int(sys.argv[1]) if len(sys.argv) > 1 else N
    t0 = time.time()
    nc = _build_program(nsteps)
    print(f"build+compile nsteps={nsteps}:", time.time() - t0)
